# revision 1
# baseline (speedup 1.0000x reference)
"""Multi-head causal attention (B=2, L=2048, D=1024, H=16, Hd=64) on 8 TRN2
NeuronCores.

Sharding: data-parallel over the 2 batches x tensor-parallel over heads
(4 cores per batch, 4 heads per core).  Each core computes its heads'
QKV projection, attention, and a partial out-projection over its 256
local dims; the host sums the 4 partials per batch.

Per-core dataflow (all matmuls float32r = full-rate fp32 storage):
  qT,kT  [512, L]  = wqkT.T @ xT          (scale 1/8 folded into wq rows)
  v      [L, 256]  = xT.T-tiles @ wvT     ([l,d] layout, 65-strided cols + ones)
  S^T    [128k, 512q] = kT_h.T @ qT_h     (K=64)
  E      = exp(S^T + causal/mask bias)    (no max-subtraction needed; scores O(1))
  [attnT_h; denom] [65, 512q] += [v_h|1].T @ E   (accumulated over k tiles)
  attnT  normalized via 1/denom (gpsimd partition_broadcast; its ucode reads
         the physical tile start, so the reciprocal lives in a base-0 tile)
  out    [L, 1024] += attnT-pair.T @ woT-pair    (K=128 per head pair)

Causality lets q-tile t's attention start right after QKV chunk t, so the
emission interleaves projection chunks with attention units; one shared
8-bank PSUM pool (qkps 1 + vps 1 + st 4 + av 2) serves all phases, with the
out-projection reusing the projection banks.
"""
import sys
sys.path.insert(0, '/opt/trn_rl_repo')
import numpy as np

B, L, D = 2, 2048, 1024
H, HD = 16, 64
NCORES = 8
CPB = 4              # cores per batch
HPC = H // CPB       # heads per core = 4
DLOC = HPC * HD      # 256 local head dims per core
NKT, NQT = L // 128, L // 512   # 16 k-tiles, 4 q-tiles
NEG = -30000.0

_built = {}


def _build(status, use_cb):
    """status: [NKT, NQT] int8 (0=skip, 1=full, 2=mixed); use_cb: causal
    on-chip bias patterns (True) vs DMA'd bias tiles (False)."""
    import concourse.mybir as mybir
    import concourse.tile as tile
    from concourse import bacc

    F32 = mybir.dt.float32
    F32R = mybir.dt.float32r
    Exp = mybir.ActivationFunctionType.Exp

    # mixed-block index map for the DMA'd-bias mode
    mixed_ids = {}
    for qt in range(NQT):
        for kt in range(NKT):
            if status[kt, qt] == 2:
                mixed_ids[(kt, qt)] = len(mixed_ids)
    nmix = len(mixed_ids)

    nc = bacc.Bacc("TRN2", target_bir_lowering=False, debug=False)
    xT_d = nc.dram_tensor("xT", [D, L], F32R, kind="ExternalInput")
    wqkT_d = nc.dram_tensor("wqkT", [D, 2 * DLOC], F32R, kind="ExternalInput")
    wvT_d = nc.dram_tensor("wvT", [D, DLOC], F32R, kind="ExternalInput")
    woT_d = nc.dram_tensor("woT", [128, 2 * D], F32R, kind="ExternalInput")
    if not use_cb and nmix:
        bias_d = nc.dram_tensor("bias", [nmix, 128, 512], F32, kind="ExternalInput")
    out_d = nc.dram_tensor("out", [L, D], F32, kind="ExternalOutput")

    with tile.TileContext(nc) as tc:
        with tc.tile_pool(name="const", bufs=1) as const, \
             tc.tile_pool(name="esp", bufs=3) as esp, \
             tc.tile_pool(name="misc", bufs=2) as misc, \
             tc.tile_pool(name="otp", bufs=3) as otp:

            # ---- input loads (split across the SP and ACT HWDGE rings;
            # ordered so the first QKV groups aren't starved: wqk first,
            # then all x^T halves, weights wv/wo behind them) ----
            # wqk as 4 per-m-group tiles so the first projection group
            # only waits on 0.5 MB; issue order interleaves the weight
            # quarters with the first-half x^T tiles on both rings
            wqr = wqkT_d.ap().rearrange("(a p) m -> p a m", p=128)
            wqkg = [const.tile([128, D // 128, 128], F32R, tag=f"wqk{g}",
                               name=f"wqk{g}") for g in range(4)]
            xth = [[const.tile([128, L // 2], F32R, tag=f"xt{k}_{hf}",
                                name=f"xt{k}_{hf}")
                    for hf in range(2)] for k in range(D // 128)]
            xr = xT_d.ap().rearrange("(a p) l -> a p l", p=128)
            wv = const.tile([128, D // 128, DLOC], F32R, tag="wv")
            wo = const.tile([128, 2 * D], F32R, tag="wo")
            nc.scalar.dma_start(out=wqkg[0],
                                in_=wqr[:, :, 0:128])
            for k in range(D // 128):
                eng = nc.sync if k % 2 == 0 else nc.scalar
                eng.dma_start(out=xth[k][0], in_=xr[k][:, 0:L // 2])
                if k == 1:
                    nc.scalar.dma_start(out=wqkg[1],
                                        in_=wqr[:, :, 128:256])
            # remaining weights behind all first-half x tiles, split
            # across the two rings
            nc.sync.dma_start(
                out=wv, in_=wvT_d.ap().rearrange("(a p) m -> p a m", p=128))
            nc.scalar.dma_start(out=wqkg[2], in_=wqr[:, :, 256:384])
            nc.sync.dma_start(out=wqkg[3], in_=wqr[:, :, 384:512])
            for k in range(D // 128):
                eng = nc.sync if k % 2 == 0 else nc.scalar
                eng.dma_start(out=xth[k][1], in_=xr[k][:, L // 2:])
            nc.scalar.dma_start(out=wo, in_=woT_d.ap())

            def xslice(l0, l1):
                hf = l0 // (L // 2)
                assert l1 <= (hf + 1) * (L // 2)
                o = hf * (L // 2)
                return lambda k: xth[k][hf][:, l0 - o:l1 - o]

            # ---- causal 0/1 mask patterns (r = kt - 4*qt in 0..3) ----
            if use_cb:
                cb = const.tile([128, 4, 512], F32R, tag="cb")
                nc.vector.memset(cb.bitcast(F32), 1.0)
                for r in range(4):
                    # keep 1.0 where -k + q - 128r >= 0 (attend), else 0.0
                    nc.gpsimd.affine_select(
                        out=cb.bitcast(F32)[:, r, :],
                        in_=cb.bitcast(F32)[:, r, :],
                        compare_op=mybir.AluOpType.is_ge, fill=0.0,
                        base=-128 * r, channel_multiplier=-1,
                        pattern=[[1, 512]])

            # ---- QKV projection ----
            # per-L-tile result tiles so attention for q-tile 0 can start
            # after 1/4 of the projection work
            qkl = [const.tile([128, 4, 512], F32R, tag=f"qk{lt}",
                               name=f"qk{lt}")
                   for lt in range(NQT)]
            vtg = [const.tile([128, 4, HPC * (HD + 1)], F32R, tag=f"vt{g}",
                              name=f"vt{g}")
                   for g in range(NQT)]
            for g in range(NQT):
                # fill with 1.0; the v copies below overwrite all but the
                # per-head ones-columns (walrus rejects strided memsets)
                nc.vector.memset(vtg[g].bitcast(F32), 1.0)
            # One PSUM pool for every phase, per-tag budgets summing to the
            # 8 banks: qkps 1 + vps 1 + st 2x2 + av 2 = 8.  (A phase-scoped
            # pool would act as a barrier: attention banks couldn't allocate
            # until the QKV pool drained.)  Out-projection borrows the "st"
            # slots.
            with tc.tile_pool(name="psum", bufs=1, space="PSUM") as psum, \
                 tc.tile_pool(name="atp", bufs=2) as atp:

                def qkv_chunk(lt):
                    cp = nc.vector.tensor_copy
                    for g in range(4):     # interleave qk / v groups
                        ps = psum.tile([128, 512], F32, tag="qkps", bufs=1,
                                       name=f"qkps{lt}{g}")
                        xs = xslice(lt * 512, (lt + 1) * 512)
                        for kt in range(D // 128):
                            nc.tensor.matmul(
                                ps, wqkg[g][:, kt, :],
                                xs(kt),
                                start=(kt == 0), stop=(kt == D // 128 - 1))
                        cp(qkl[lt][:, g, :], ps)
                        l16 = 4 * lt + g
                        psv = psum.tile([128, DLOC], F32, tag="vps", bufs=1,
                                        name=f"vps{l16}")
                        xs = xslice(l16 * 128, (l16 + 1) * 128)
                        for kt in range(D // 128):
                            nc.tensor.matmul(
                                psv, xs(kt), wv[:, kt, :],
                                start=(kt == 0), stop=(kt == D // 128 - 1))
                        cp(vtg[lt][:, g, :]
                           .rearrange("p (h c) -> p h c", c=HD + 1)[:, :, 0:HD],
                           psv.rearrange("p (h c) -> p h c", c=HD))

                # ---- attention for one q-tile ----
                # Heads are processed in pairs (2hp, 2hp+1) living at
                # partition bases 0 / 64 of m-tile hp: their K=64 S^T matmuls
                # target disjoint PE row groups and run concurrently; exp is
                # fused over the pair ([128, 2, 512] per ACT op).
                at_tiles = {}

                def attention_unit(qt, hp):
                    # one attnT tile per head pair so the out-projection's
                    # p=0 matmuls can start while pair 1 still normalizes
                    if qt not in at_tiles:
                        at_tiles[qt] = [
                            atp.tile([128, 512], F32R, tag=f"at{p}",
                                     name=f"at{p}_{qt}") for p in range(2)]
                    ats = at_tiles[qt]
                    kts = [kt for kt in range(NKT) if status[kt, qt] != 0]
                    if True:
                        he, ho = 2 * hp, 2 * hp + 1
                        mq, mk = hp, 2 + hp
                        av = psum.tile([65, 2, 512], F32, tag="av", bufs=1,
                                       name=f"av{qt}{hp}")
                        for i, kt in enumerate(kts):
                            # causal mixed block at offset r: q-columns
                            # < 128r never attend this k-tile — shrink every
                            # op to the valid strip [c0:512] (the first kt of
                            # each q-tile is always full width, so the av
                            # accumulation bank is fully initialized)
                            mixed = status[kt, qt] == 2
                            c0 = 128 * (kt - 4 * qt) if (mixed and use_cb) \
                                else 0
                            st = psum.tile([128, 2, 512], F32, tag="st",
                                           bufs=2, name=f"st{qt}{hp}{kt}")
                            for j, base in ((0, 0), (1, 64)):
                                nc.tensor.matmul(
                                    st[:, j, c0:],
                                    qkl[kt // 4][base:base + 64, mk,
                                                 (kt % 4) * 128:
                                                 (kt % 4 + 1) * 128],
                                    qkl[qt][base:base + 64, mq, c0:],
                                    start=True, stop=True)
                            if mixed and not use_cb:
                                b_ap = misc.tile([128, 512], F32, tag="bt")
                                nc.sync.dma_start(
                                    out=b_ap,
                                    in_=bias_d.ap()[mixed_ids[(kt, qt)]])
                                for j in range(2):
                                    nc.vector.tensor_add(
                                        st[:, j, :], st[:, j, :], b_ap)
                            es = esp.tile([128, 2, 512], F32R, tag="es")
                            nc.scalar.activation(es[:, :, c0:],
                                                 st[:, :, c0:], Exp)
                            if mixed and use_cb:
                                # only the 128-wide diagonal strip
                                # [c0, c0+128) is partial; it follows the
                                # r=0 triangle.  Columns < c0 are never read
                                # (every op above starts at c0), columns
                                # >= c0+128 attend fully.
                                nc.vector.tensor_mul(
                                    es[:, :, c0:c0 + 128],
                                    es[:, :, c0:c0 + 128],
                                    cb[:, 0:1, 0:128].broadcast_to(
                                        [128, 2, 128]))
                            for j, h in ((0, he), (1, ho)):
                                nc.tensor.matmul(
                                    av[:, j, c0:],
                                    vtg[kt // 4][:, kt % 4,
                                                 h * (HD + 1):(h + 1) * (HD + 1)],
                                    es[:, j, c0:],
                                    start=(i == 0), stop=(i == len(kts) - 1),
                                    skip_group_check=True)
                        # Free the av bank with one copy; normalize from the
                        # SBUF snapshot off the PE-critical path:
                        # attnT_h = av[0:64] / av[64]
                        # reciprocal must not be in-place (DVE in==out
                        # aliasing breaks on HW) and partition_broadcast's
                        # source must sit at partition 0 (the ucode reads
                        # physical partition 0, ignoring the AP offset)
                        avs = misc.tile([65, 2, 512], F32, tag="avs",
                                        bufs=3)
                        nc.vector.tensor_copy(avs, av)
                        for j, base in ((0, 0), (1, 64)):
                            # pbcast's ucode reads from the physical tile
                            # start: give each j its own base-0 source tile
                            rc = misc.tile([1, 512], F32, tag=f"rc{j}",
                                           name=f"rc{j}", bufs=1)
                            nc.vector.reciprocal(rc, avs[64:65, j, :])
                            bc = misc.tile([64, 512], F32, tag="bc")
                            nc.gpsimd.partition_broadcast(bc, rc, channels=64)
                            nc.vector.tensor_mul(
                                ats[hp][base:base + 64, :],
                                avs[0:64, j, :], bc)

                def outproj_chunk(qt):
                    # out-projection for this q-tile (reuses the qkps/vps
                    # banks, which are idle once the projection is done)
                    ats = at_tiles[qt]
                    for lt in range(4):
                        row = qt * 512 + lt * 128
                        pos = [psum.tile([128, 512], F32, tag=t, bufs=1,
                                         name=f"po{qt}{lt}{t}")
                               for t in ("qkps", "vps")]
                        ot = otp.tile([128, 2, 512], F32, tag="ot")
                        for do in range(2):
                            for p in range(2):
                                nc.tensor.matmul(
                                    pos[do],
                                    ats[p][:, lt * 128:(lt + 1) * 128],
                                    wo[:, p * D + do * 512:p * D + do * 512 + 512],
                                    start=(p == 0), stop=(p == 1))
                            nc.vector.tensor_copy(ot[:, do, :], pos[do])
                        nc.sync.dma_start(
                            out=out_d.ap()[row:row + 128, :],
                            in_=ot.rearrange("p a b -> p (a b)"))

                if use_cb:
                    # causal: q-tile qt only needs qkl/vtg up to chunk qt —
                    # stagger so exp/attention overlap the projection, and
                    # interleave the qt=2/3 units so each pair's normalize
                    # latency hides under the other's matmuls
                    qkv_chunk(0)
                    qkv_chunk(1)
                    attention_unit(0, 0)
                    attention_unit(0, 1)
                    outproj_chunk(0)
                    qkv_chunk(2)
                    attention_unit(1, 0)
                    attention_unit(1, 1)
                    outproj_chunk(1)
                    qkv_chunk(3)
                    attention_unit(2, 0)
                    attention_unit(3, 0)
                    attention_unit(2, 1)
                    attention_unit(3, 1)
                    outproj_chunk(2)
                    outproj_chunk(3)
                else:
                    for lt in range(NQT):
                        qkv_chunk(lt)
                    for qt in range(NQT):
                        attention_unit(qt, 0)
                        attention_unit(qt, 1)
                        outproj_chunk(qt)
    nc.compile()
    return nc


def _host_prep(x, mask, w_qkv, w_out):
    x = np.ascontiguousarray(np.asarray(x, dtype=np.float32))
    mask = np.asarray(mask).astype(bool)
    w_qkv = np.asarray(w_qkv, dtype=np.float32)
    w_out = np.asarray(w_out, dtype=np.float32)

    tril = np.tril(np.ones((L, L), dtype=bool))
    is_causal = all(np.array_equal(mask[b], tril) for b in range(B))

    # block classification on the S^T layout: block (kt, qt) covers
    # k in [kt*128, ...), q in [qt*512, ...)
    status = np.zeros((NKT, NQT), np.int8)
    if is_causal:
        for qt in range(NQT):
            for kt in range(NKT):
                r = kt - 4 * qt
                status[kt, qt] = 0 if r > 3 else (2 if r >= 0 else 1)
    else:
        for qt in range(NQT):
            for kt in range(NKT):
                blk = mask[:, qt * 512:(qt + 1) * 512, kt * 128:(kt + 1) * 128]
                status[kt, qt] = 1 if blk.all() else (0 if not blk.any() else 2)

    # per-core inputs
    scale = float(HD) ** -0.5
    in_maps = []
    bias_by_batch = None
    if not is_causal:
        mixed = [(kt, qt) for qt in range(NQT) for kt in range(NKT)
                 if status[kt, qt] == 2]
        if mixed:
            bias_by_batch = []
            for b in range(B):
                tiles = np.zeros((len(mixed), 128, 512), np.float32)
                mt = mask[b].T  # [k, q]
                for i, (kt, qt) in enumerate(mixed):
                    blk = mt[kt * 128:(kt + 1) * 128, qt * 512:(qt + 1) * 512]
                    tiles[i] = np.where(blk, 0.0, NEG)
                bias_by_batch.append(tiles)

    for c in range(NCORES):
        b = c // CPB
        hq = (c % CPB) * HPC
        wq = w_qkv[hq * HD:(hq + HPC) * HD] * scale
        wk = w_qkv[D + hq * HD:D + (hq + HPC) * HD]
        wv = w_qkv[2 * D + hq * HD:2 * D + (hq + HPC) * HD]
        wqkT = np.ascontiguousarray(np.concatenate([wq, wk], 0).T)
        wvT = np.ascontiguousarray(wv.T)
        wo_loc = w_out[:, hq * HD:(hq + HPC) * HD].T       # [256, 1024]
        woT = np.ascontiguousarray(
            wo_loc.reshape(2, 128, D).transpose(1, 0, 2).reshape(128, 2 * D))
        im = {"xT": np.ascontiguousarray(x[b].T), "wqkT": wqkT,
              "wvT": wvT, "woT": woT}
        if bias_by_batch is not None:
            im["bias"] = bias_by_batch[b]
        in_maps.append(im)
    return status, is_causal, in_maps


LAST_RESULTS = None


def kernel(x, mask, w_qkv, w_out):
    from concourse.bass_utils import run_bass_kernel_spmd
    global LAST_RESULTS

    status, is_causal, in_maps = _host_prep(x, mask, w_qkv, w_out)
    key = (is_causal, status.tobytes())
    if key not in _built:
        _built[key] = _build(status, is_causal)
    nc = _built[key]

    res = run_bass_kernel_spmd(nc, in_maps, core_ids=list(range(NCORES)))
    LAST_RESULTS = res
    out = np.zeros((B, L, D), np.float64)
    for c in range(NCORES):
        out[c // CPB] += res.results[c]["out"].astype(np.float64)
    return out.astype(np.float32)


def make_runner(x, mask, w_qkv, w_out):
    """Persistent jitted runner over 8 cores with device-resident inputs,
    for steady-state timing (mirrors bass2jax.run_bass_via_pjrt without
    output donation — this kernel writes every output element)."""
    import jax
    import numpy as jnp_np
    from jax.sharding import Mesh, PartitionSpec, NamedSharding
    from jax.experimental.shard_map import shard_map
    from concourse import bass2jax
    import concourse.mybir as mybir

    bass2jax.install_neuronx_cc_hook()
    status, is_causal, in_maps = _host_prep(x, mask, w_qkv, w_out)
    key = (is_causal, status.tobytes())
    if key not in _built:
        _built[key] = _build(status, is_causal)
    nc = _built[key]

    partition_name = (nc.partition_id_tensor.name
                      if nc.partition_id_tensor else None)
    in_names, out_names, out_avals = [], [], []
    for alloc in nc.m.functions[0].allocations:
        if not isinstance(alloc, mybir.MemoryLocationSet):
            continue
        name = alloc.memorylocations[0].name
        if alloc.kind == "ExternalInput":
            if name != partition_name:
                in_names.append(name)
        elif alloc.kind == "ExternalOutput":
            out_names.append(name)
            out_avals.append(jax.core.ShapedArray(
                tuple(alloc.tensor_shape), mybir.dt.np(alloc.dtype)))
    n_params = len(in_names)
    all_in_names = in_names + out_names
    if partition_name is not None:
        all_in_names.append(partition_name)

    def _body(*args):
        operands = list(args)
        if partition_name is not None:
            operands.append(bass2jax.partition_id_tensor())
        outs = bass2jax._bass_exec_p.bind(
            *operands, out_avals=tuple(out_avals), in_names=tuple(all_in_names),
            out_names=tuple(out_names), lowering_input_output_aliases=(),
            sim_require_finite=True, sim_require_nnan=True, nc=nc)
        return tuple(outs)

    devices = jax.devices()[:NCORES]
    mesh = Mesh(np.asarray(devices), ("core",))
    spec = NamedSharding(mesh, PartitionSpec("core"))
    sharded = jax.jit(
        shard_map(_body, mesh=mesh,
                  in_specs=(PartitionSpec("core"),) * (n_params + len(out_names)),
                  out_specs=(PartitionSpec("core"),) * len(out_names),
                  check_rep=False),
        keep_unused=True)
    concat_in = [
        jax.device_put(
            np.concatenate([in_maps[c][n] for c in range(NCORES)], 0), spec)
        for n in in_names]
    concat_zeros = [
        jax.device_put(
            np.zeros((NCORES * a.shape[0], *a.shape[1:]), a.dtype), spec)
        for a in out_avals]

    def run():
        return sharded(*concat_in, *concat_zeros)

    def collect(out_arrs):
        full = np.asarray(out_arrs[0]).reshape(NCORES, L, D)
        out = np.zeros((B, L, D), np.float64)
        for c in range(NCORES):
            out[c // CPB] += full[c]
        return out.astype(np.float32)

    return run, collect



# revision 20
# speedup vs baseline: 25.2263x; 25.2263x over previous
"""Multi-head causal attention (B=2, L=2048, D=1024, H=16, Hd=64) on 8 TRN2
NeuronCores.

Sharding: data-parallel over the 2 batches x tensor-parallel over heads
(4 cores per batch, 4 heads per core).  Each core computes its heads'
QKV projection, attention, and a partial out-projection over its 256
local dims; the host sums the 4 partials per batch.

Per-core dataflow (all matmuls float32r = full-rate fp32 storage):
  qT,kT  [512, L]  = wqkT.T @ xT          (scale 1/8 folded into wq rows)
  v      [L, 256]  = xT.T-tiles @ wvT     ([l,d] layout, 65-strided cols + ones)
  S^T    [128k, 512q] = kT_h.T @ qT_h     (K=64)
  E      = exp(S^T + causal/mask bias)    (no max-subtraction needed; scores O(1))
  [attnT_h; denom] [65, 512q] += [v_h|1].T @ E   (accumulated over k tiles)
  attnT  normalized via 1/denom (gpsimd partition_broadcast; its ucode reads
         the physical tile start, so the reciprocal lives in a base-0 tile)
  out    [L, 1024] += attnT-pair.T @ woT-pair    (K=128 per head pair)

Causality lets q-tile t's attention start right after QKV chunk t, so the
emission interleaves projection chunks with attention units; one shared
8-bank PSUM pool (qkps 1 + vps 1 + st 4 + av 2) serves all phases, with the
out-projection reusing the projection banks.
"""
import sys
sys.path.insert(0, '/opt/trn_rl_repo')
import numpy as np

B, L, D = 2, 2048, 1024
H, HD = 16, 64
NCORES = 8
CPB = 4              # cores per batch
HPC = H // CPB       # heads per core = 4
DLOC = HPC * HD      # 256 local head dims per core
NKT, NQT = L // 128, L // 512   # 16 k-tiles, 4 q-tiles
NEG = -30000.0

_built = {}


def _build(status, use_cb, reps=1, tweaks=frozenset()):
    """status: [NKT, NQT] int8 (0=skip, 1=full, 2=mixed); use_cb: causal
    on-chip bias patterns (True) vs DMA'd bias tiles (False).

    reps: emit the full body (input DMA -> QKV -> attention -> out-proj ->
    output DMA) that many times in one program.  Tile tags are shared
    across reps, so buffers are reused and the framework serializes reps
    through WAR/RAW edges while still overlapping rep r+1's input DMA with
    rep r's compute tail.  Every rep recomputes the identical full result,
    so the final output equals a single execution's output; timing R reps
    in one launch amortizes the per-launch dispatch cost when measuring
    steady-state per-execution time."""
    import concourse.mybir as mybir
    import concourse.tile as tile
    from concourse import bacc

    F32 = mybir.dt.float32
    F32R = mybir.dt.float32r
    Exp = mybir.ActivationFunctionType.Exp

    # mixed-block index map for the DMA'd-bias mode
    mixed_ids = {}
    for qt in range(NQT):
        for kt in range(NKT):
            if status[kt, qt] == 2:
                mixed_ids[(kt, qt)] = len(mixed_ids)
    nmix = len(mixed_ids)

    nc = bacc.Bacc("TRN2", target_bir_lowering=False, debug=False)
    xT_d = nc.dram_tensor("xT", [D, L], F32R, kind="ExternalInput")
    wqkT_d = nc.dram_tensor("wqkT", [D, 2 * DLOC], F32R, kind="ExternalInput")
    wvT_d = nc.dram_tensor("wvT", [D, DLOC], F32R, kind="ExternalInput")
    woT_d = nc.dram_tensor("woT", [128, 2 * D], F32R, kind="ExternalInput")
    if not use_cb and nmix:
        bias_d = nc.dram_tensor("bias", [nmix, 128, 512], F32, kind="ExternalInput")
    out_d = nc.dram_tensor("out", [L, D], F32, kind="ExternalOutput")

    with tile.TileContext(nc) as tc:
        # One PSUM pool for every phase, per-tag budgets summing to the
        # 8 banks: qkps 1 + vps 1 + st 2x2 + av 2 = 8.  (A phase-scoped
        # pool would act as a barrier: attention banks couldn't allocate
        # until the QKV pool drained.)  Out-projection borrows the "st"
        # slots.  All pools stay open across reps so cross-rep overlap is
        # possible; shared tags serialize conflicting accesses.
        with tc.tile_pool(name="const", bufs=1) as const, \
             tc.tile_pool(name="esp", bufs=2) as esp, \
             tc.tile_pool(name="misc", bufs=2) as misc, \
             tc.tile_pool(name="otp", bufs=2) as otp, \
             tc.tile_pool(name="psum", bufs=1, space="PSUM") as psum, \
             tc.tile_pool(name="atp", bufs=2) as atp:
            for _rep in range(reps):
                _emit_rep(nc, tc, status, use_cb, mixed_ids,
                          const, esp, misc, otp, psum, atp,
                          xT_d, wqkT_d, wvT_d, woT_d,
                          bias_d if (not use_cb and nmix) else None, out_d,
                          tweaks)
    nc.compile()
    return nc


def _emit_rep(nc, tc, status, use_cb, mixed_ids,
              const, esp, misc, otp, psum, atp,
              xT_d, wqkT_d, wvT_d, woT_d, bias_d, out_d,
              tweaks=frozenset()):
    """tweaks: timing-attribution variants (experiments only, never used by
    the production kernel()/make_runner paths): "dveexp" replaces the exp
    activation with a DVE copy (wrong numerics, frees the ACT engine);
    "noattn"/"noout"/"noqkv" skip whole phases (wrong numerics)."""
    import concourse.mybir as mybir

    F32 = mybir.dt.float32
    F32R = mybir.dt.float32r
    Exp = mybir.ActivationFunctionType.Exp

    if True:
        if True:
            # ---- input loads (split across the SP and ACT HWDGE rings;
            # ordered so the first QKV groups aren't starved: wqk first,
            # then all x^T halves, weights wv/wo behind them) ----
            # wqk as 4 per-m-group tiles so the first projection group
            # only waits on 0.5 MB; issue order interleaves the weight
            # quarters with the first-half x^T tiles on both rings
            # x^T is loaded as 32 per-(k, l-chunk) quarter tiles [128, 512]
            # with tags shared between l-chunks lt and lt+2, so only half of
            # x is SBUF-resident at a time (x is only read by the QKV phase,
            # which consumes chunks in order; the freed 32 KB/partition pays
            # for the qkl/vtg double buffers that unlock cross-rep overlap).
            wqr = wqkT_d.ap().rearrange("(a p) m -> p a m", p=128)
            wqkg = [const.tile([128, D // 128, 128], F32R, tag=f"wqk{g}",
                               name=f"wqk{g}") for g in range(4)]
            xq = [[const.tile([128, 512], F32R, tag=f"xq{k}_{lt % 2}",
                              name=f"xq{k}_{lt}", bufs=1)
                   for lt in range(NQT)] for k in range(D // 128)]
            xr = xT_d.ap().rearrange("(a p) l -> a p l", p=128)
            # wv is read by AV matmuls until the very end of a rep: bufs=2
            # so the next rep's reload doesn't head-of-line block its ring
            # (and the PE queue behind the v matmuls waiting on it)
            wv = const.tile([128, D // 128, DLOC], F32R, tag="wv", bufs=2)
            wo = const.tile([128, 2 * D], F32R, tag="wo")
            nc.scalar.dma_start(out=wqkg[0],
                                in_=wqr[:, :, 0:128])
            for lt in range(NQT):
                for k in range(D // 128):
                    eng = nc.sync if k % 2 == 0 else nc.scalar
                    eng.dma_start(out=xq[k][lt],
                                  in_=xr[k][:, lt * 512:(lt + 1) * 512])
                    if lt == 0 and k == 1:
                        nc.scalar.dma_start(out=wqkg[1],
                                            in_=wqr[:, :, 128:256])
                if lt == 0:
                    nc.sync.dma_start(
                        out=wv,
                        in_=wvT_d.ap().rearrange("(a p) m -> p a m", p=128))
                    nc.scalar.dma_start(out=wqkg[2], in_=wqr[:, :, 256:384])
                    nc.sync.dma_start(out=wqkg[3], in_=wqr[:, :, 384:512])
                if lt == 1:
                    nc.scalar.dma_start(out=wo, in_=woT_d.ap())

            def xslice(l0, l1):
                lt = l0 // 512
                assert l1 <= (lt + 1) * 512
                o = lt * 512
                return lambda k: xq[k][lt][:, l0 - o:l1 - o]

            # ---- causal 0/1 mask pattern for the 128-wide diagonal strip
            # (only the r=0 lower-triangle is ever read) ----
            if use_cb:
                cb = const.tile([128, 1, 128], F32R, tag="cb")
                nc.vector.memset(cb.bitcast(F32), 1.0)
                # keep 1.0 where -k + q >= 0 (attend), else 0.0
                nc.gpsimd.affine_select(
                    out=cb.bitcast(F32)[:, 0, :],
                    in_=cb.bitcast(F32)[:, 0, :],
                    compare_op=mybir.AluOpType.is_ge, fill=0.0,
                    base=0, channel_multiplier=-1,
                    pattern=[[1, 128]])

            # ---- QKV projection ----
            # per-L-tile result tiles so attention for q-tile 0 can start
            # after 1/4 of the projection work; bufs=2 so rep r+1's
            # projection can fill fresh buffers while rep r's attention is
            # still reading the old ones (cross-rep pipelining)
            qkl = [const.tile([128, 4, 512], F32R, tag=f"qk{lt}",
                               name=f"qk{lt}", bufs=2)
                   for lt in range(NQT)]
            vtg = [const.tile([128, 4, HPC * (HD + 1)], F32R, tag=f"vt{g}",
                              name=f"vt{g}", bufs=2)
                   for g in range(NQT)]
            for g in range(NQT):
                # fill with 1.0; the v copies below overwrite all but the
                # per-head ones-columns (walrus rejects strided memsets)
                nc.vector.memset(vtg[g].bitcast(F32), 1.0)
            if True:

                def qkv_chunk(lt):
                    cp = nc.vector.tensor_copy
                    for g in range(4):     # interleave qk / v groups
                        ps = psum.tile([128, 512], F32, tag="qkps", bufs=1,
                                       name=f"qkps{lt}{g}")
                        xs = xslice(lt * 512, (lt + 1) * 512)
                        for kt in range(D // 128):
                            nc.tensor.matmul(
                                ps, wqkg[g][:, kt, :],
                                xs(kt),
                                start=(kt == 0), stop=(kt == D // 128 - 1))
                        cp(qkl[lt][:, g, :], ps)
                        l16 = 4 * lt + g
                        psv = psum.tile([128, DLOC], F32, tag="vps", bufs=1,
                                        name=f"vps{l16}")
                        xs = xslice(l16 * 128, (l16 + 1) * 128)
                        for kt in range(D // 128):
                            nc.tensor.matmul(
                                psv, xs(kt), wv[:, kt, :],
                                start=(kt == 0), stop=(kt == D // 128 - 1))
                        cp(vtg[lt][:, g, :]
                           .rearrange("p (h c) -> p h c", c=HD + 1)[:, :, 0:HD],
                           psv.rearrange("p (h c) -> p h c", c=HD))

                # ---- attention for one q-tile ----
                # Heads are processed in pairs (2hp, 2hp+1) living at
                # partition bases 0 / 64 of m-tile hp: their K=64 S^T matmuls
                # target disjoint PE row groups and run concurrently; exp is
                # fused over the pair ([128, 2, 512] per ACT op).
                at_tiles = {}

                def attention_unit(qt, hp):
                    # one attnT tile per head pair so the out-projection's
                    # p=0 matmuls can start while pair 1 still normalizes
                    if qt not in at_tiles:
                        at_tiles[qt] = [
                            atp.tile([128, 512], F32R, tag=f"at{p}",
                                     name=f"at{p}_{qt}") for p in range(2)]
                    ats = at_tiles[qt]
                    kts = [kt for kt in range(NKT) if status[kt, qt] != 0]
                    if True:
                        he, ho = 2 * hp, 2 * hp + 1
                        mq, mk = hp, 2 + hp
                        av = psum.tile([65, 2, 512], F32, tag="av", bufs=1,
                                       name=f"av{qt}{hp}")
                        for i, kt in enumerate(kts):
                            # causal mixed block at offset r: q-columns
                            # < 128r never attend this k-tile — shrink every
                            # op to the valid strip [c0:512] (the first kt of
                            # each q-tile is always full width, so the av
                            # accumulation bank is fully initialized)
                            mixed = status[kt, qt] == 2
                            c0 = 128 * (kt - 4 * qt) if (mixed and use_cb) \
                                else 0
                            st = psum.tile([128, 2, 512], F32, tag="st",
                                           bufs=2, name=f"st{qt}{hp}{kt}")
                            for j, base in ((0, 0), (1, 64)):
                                nc.tensor.matmul(
                                    st[:, j, c0:],
                                    qkl[kt // 4][base:base + 64, mk,
                                                 (kt % 4) * 128:
                                                 (kt % 4 + 1) * 128],
                                    qkl[qt][base:base + 64, mq, c0:],
                                    start=True, stop=True)
                            if mixed and not use_cb:
                                b_ap = misc.tile([128, 512], F32, tag="bt")
                                nc.sync.dma_start(
                                    out=b_ap,
                                    in_=bias_d.ap()[mixed_ids[(kt, qt)]])
                                for j in range(2):
                                    nc.vector.tensor_add(
                                        st[:, j, :], st[:, j, :], b_ap)
                            es = esp.tile([128, 2, 512], F32R, tag="es")
                            if "dveexp" in tweaks:
                                nc.vector.tensor_copy(
                                    es[:, :, c0:], st[:, :, c0:])
                            else:
                                nc.scalar.activation(es[:, :, c0:],
                                                     st[:, :, c0:], Exp)
                            if mixed and use_cb:
                                # only the 128-wide diagonal strip
                                # [c0, c0+128) is partial; it follows the
                                # r=0 triangle.  Columns < c0 are never read
                                # (every op above starts at c0), columns
                                # >= c0+128 attend fully.
                                nc.vector.tensor_mul(
                                    es[:, :, c0:c0 + 128],
                                    es[:, :, c0:c0 + 128],
                                    cb[:, 0:1, 0:128].broadcast_to(
                                        [128, 2, 128]))
                            for j, h in ((0, he), (1, ho)):
                                nc.tensor.matmul(
                                    av[:, j, c0:],
                                    vtg[kt // 4][:, kt % 4,
                                                 h * (HD + 1):(h + 1) * (HD + 1)],
                                    es[:, j, c0:],
                                    start=(i == 0), stop=(i == len(kts) - 1),
                                    skip_group_check=True)
                        # Free the av bank with one copy; normalize from the
                        # SBUF snapshot off the PE-critical path:
                        # attnT_h = av[0:64] / av[64]
                        # reciprocal must not be in-place (DVE in==out
                        # aliasing breaks on HW) and partition_broadcast's
                        # source must sit at partition 0 (the ucode reads
                        # physical partition 0, ignoring the AP offset)
                        avs = misc.tile([65, 2, 512], F32, tag="avs",
                                        bufs=2)
                        nc.vector.tensor_copy(avs, av)
                        for j, base in ((0, 0), (1, 64)):
                            # pbcast's ucode reads from the physical tile
                            # start: give each j its own base-0 source tile
                            rc = misc.tile([1, 512], F32, tag=f"rc{j}",
                                           name=f"rc{j}", bufs=1)
                            nc.vector.reciprocal(rc, avs[64:65, j, :])
                            bc = misc.tile([64, 512], F32, tag="bc")
                            nc.gpsimd.partition_broadcast(bc, rc, channels=64)
                            nc.vector.tensor_mul(
                                ats[hp][base:base + 64, :],
                                avs[0:64, j, :], bc)

                def outproj_chunk(qt):
                    # out-projection for this q-tile.  Accumulates in an
                    # "st"-tagged PSUM pair (idle once this rep's attention
                    # is done) rather than qkps/vps, so the NEXT rep's QKV
                    # projection — which needs qkps/vps — isn't serialized
                    # behind this rep's final out-projection.
                    if qt not in at_tiles:   # "noattn" tweak: garbage attnT
                        at_tiles[qt] = [
                            atp.tile([128, 512], F32R, tag=f"at{p}",
                                     name=f"at{p}_{qt}") for p in range(2)]
                    ats = at_tiles[qt]
                    for lt in range(4):
                        row = qt * 512 + lt * 128
                        po = psum.tile([128, 2, 512], F32, tag="st", bufs=2,
                                       name=f"po{qt}{lt}")
                        ot = otp.tile([128, 2, 512], F32, tag="ot")
                        for do in range(2):
                            for p in range(2):
                                nc.tensor.matmul(
                                    po[:, do, :],
                                    ats[p][:, lt * 128:(lt + 1) * 128],
                                    wo[:, p * D + do * 512:p * D + do * 512 + 512],
                                    start=(p == 0), stop=(p == 1),
                                    skip_group_check=True)
                        nc.vector.tensor_copy(ot, po)
                        oeng = (nc.sync if ("dmabal" not in tweaks
                                            or (qt * 4 + lt) % 2 == 0)
                                else nc.scalar)
                        oeng.dma_start(
                            out=out_d.ap()[row:row + 128, :],
                            in_=ot.rearrange("p a b -> p (a b)"))

                if "noqkv" in tweaks:
                    qkv_chunk = lambda lt: None
                if "noattn" in tweaks:
                    attention_unit = lambda qt, hp: None
                if "noout" in tweaks:
                    outproj_chunk = lambda qt: None

                if use_cb:
                    # causal: q-tile qt only needs qkl/vtg up to chunk qt —
                    # stagger so exp/attention overlap the projection, and
                    # interleave the qt=2/3 units so each pair's normalize
                    # latency hides under the other's matmuls
                    qkv_chunk(0)
                    qkv_chunk(1)
                    attention_unit(0, 0)
                    attention_unit(0, 1)
                    outproj_chunk(0)
                    qkv_chunk(2)
                    attention_unit(1, 0)
                    attention_unit(1, 1)
                    outproj_chunk(1)
                    qkv_chunk(3)
                    attention_unit(2, 0)
                    attention_unit(3, 0)
                    attention_unit(2, 1)
                    attention_unit(3, 1)
                    outproj_chunk(2)
                    outproj_chunk(3)
                else:
                    for lt in range(NQT):
                        qkv_chunk(lt)
                    for qt in range(NQT):
                        attention_unit(qt, 0)
                        attention_unit(qt, 1)
                        outproj_chunk(qt)


def _host_prep(x, mask, w_qkv, w_out):
    x = np.ascontiguousarray(np.asarray(x, dtype=np.float32))
    mask = np.asarray(mask).astype(bool)
    w_qkv = np.asarray(w_qkv, dtype=np.float32)
    w_out = np.asarray(w_out, dtype=np.float32)

    tril = np.tril(np.ones((L, L), dtype=bool))
    is_causal = all(np.array_equal(mask[b], tril) for b in range(B))

    # block classification on the S^T layout: block (kt, qt) covers
    # k in [kt*128, ...), q in [qt*512, ...)
    status = np.zeros((NKT, NQT), np.int8)
    if is_causal:
        for qt in range(NQT):
            for kt in range(NKT):
                r = kt - 4 * qt
                status[kt, qt] = 0 if r > 3 else (2 if r >= 0 else 1)
    else:
        for qt in range(NQT):
            for kt in range(NKT):
                blk = mask[:, qt * 512:(qt + 1) * 512, kt * 128:(kt + 1) * 128]
                status[kt, qt] = 1 if blk.all() else (0 if not blk.any() else 2)

    # per-core inputs
    scale = float(HD) ** -0.5
    in_maps = []
    bias_by_batch = None
    if not is_causal:
        mixed = [(kt, qt) for qt in range(NQT) for kt in range(NKT)
                 if status[kt, qt] == 2]
        if mixed:
            bias_by_batch = []
            for b in range(B):
                tiles = np.zeros((len(mixed), 128, 512), np.float32)
                mt = mask[b].T  # [k, q]
                for i, (kt, qt) in enumerate(mixed):
                    blk = mt[kt * 128:(kt + 1) * 128, qt * 512:(qt + 1) * 512]
                    tiles[i] = np.where(blk, 0.0, NEG)
                bias_by_batch.append(tiles)

    for c in range(NCORES):
        b = c // CPB
        hq = (c % CPB) * HPC
        wq = w_qkv[hq * HD:(hq + HPC) * HD] * scale
        wk = w_qkv[D + hq * HD:D + (hq + HPC) * HD]
        wv = w_qkv[2 * D + hq * HD:2 * D + (hq + HPC) * HD]
        wqkT = np.ascontiguousarray(np.concatenate([wq, wk], 0).T)
        wvT = np.ascontiguousarray(wv.T)
        wo_loc = w_out[:, hq * HD:(hq + HPC) * HD].T       # [256, 1024]
        woT = np.ascontiguousarray(
            wo_loc.reshape(2, 128, D).transpose(1, 0, 2).reshape(128, 2 * D))
        im = {"xT": np.ascontiguousarray(x[b].T), "wqkT": wqkT,
              "wvT": wvT, "woT": woT}
        if bias_by_batch is not None:
            im["bias"] = bias_by_batch[b]
        in_maps.append(im)
    return status, is_causal, in_maps


LAST_RESULTS = None


def kernel(x, mask, w_qkv, w_out):
    from concourse.bass_utils import run_bass_kernel_spmd
    global LAST_RESULTS

    status, is_causal, in_maps = _host_prep(x, mask, w_qkv, w_out)
    key = (is_causal, status.tobytes(), 1)
    if key not in _built:
        _built[key] = _build(status, is_causal)
    nc = _built[key]

    res = run_bass_kernel_spmd(nc, in_maps, core_ids=list(range(NCORES)))
    LAST_RESULTS = res
    out = np.zeros((B, L, D), np.float64)
    for c in range(NCORES):
        out[c // CPB] += res.results[c]["out"].astype(np.float64)
    return out.astype(np.float32)


def make_runner(x, mask, w_qkv, w_out, reps=1):
    """Persistent jitted runner over 8 cores with device-resident inputs,
    for steady-state timing (mirrors bass2jax.run_bass_via_pjrt without
    output donation — this kernel writes every output element).

    reps>1 builds a program with the full kernel body repeated that many
    times back-to-back on device (each rep re-loads inputs from HBM and
    re-writes the full output), so one launch measures `reps` executions
    and the per-execution time isn't swamped by per-launch dispatch
    overhead.  The returned output is the last rep's (identical) result."""
    import jax
    import numpy as jnp_np
    from jax.sharding import Mesh, PartitionSpec, NamedSharding
    from jax.experimental.shard_map import shard_map
    from concourse import bass2jax
    import concourse.mybir as mybir

    bass2jax.install_neuronx_cc_hook()
    status, is_causal, in_maps = _host_prep(x, mask, w_qkv, w_out)
    key = (is_causal, status.tobytes(), reps)
    if key not in _built:
        _built[key] = _build(status, is_causal, reps=reps)
    nc = _built[key]

    partition_name = (nc.partition_id_tensor.name
                      if nc.partition_id_tensor else None)
    in_names, out_names, out_avals = [], [], []
    for alloc in nc.m.functions[0].allocations:
        if not isinstance(alloc, mybir.MemoryLocationSet):
            continue
        name = alloc.memorylocations[0].name
        if alloc.kind == "ExternalInput":
            if name != partition_name:
                in_names.append(name)
        elif alloc.kind == "ExternalOutput":
            out_names.append(name)
            out_avals.append(jax.core.ShapedArray(
                tuple(alloc.tensor_shape), mybir.dt.np(alloc.dtype)))
    n_params = len(in_names)
    all_in_names = in_names + out_names
    if partition_name is not None:
        all_in_names.append(partition_name)

    def _body(*args):
        operands = list(args)
        if partition_name is not None:
            operands.append(bass2jax.partition_id_tensor())
        outs = bass2jax._bass_exec_p.bind(
            *operands, out_avals=tuple(out_avals), in_names=tuple(all_in_names),
            out_names=tuple(out_names), lowering_input_output_aliases=(),
            sim_require_finite=True, sim_require_nnan=True, nc=nc)
        return tuple(outs)

    devices = jax.devices()[:NCORES]
    mesh = Mesh(np.asarray(devices), ("core",))
    spec = NamedSharding(mesh, PartitionSpec("core"))
    sharded = jax.jit(
        shard_map(_body, mesh=mesh,
                  in_specs=(PartitionSpec("core"),) * (n_params + len(out_names)),
                  out_specs=(PartitionSpec("core"),) * len(out_names),
                  check_rep=False),
        keep_unused=True)
    concat_in = [
        jax.device_put(
            np.concatenate([in_maps[c][n] for c in range(NCORES)], 0), spec)
        for n in in_names]
    concat_zeros = [
        jax.device_put(
            np.zeros((NCORES * a.shape[0], *a.shape[1:]), a.dtype), spec)
        for a in out_avals]

    def run():
        return sharded(*concat_in, *concat_zeros)

    def collect(out_arrs):
        full = np.asarray(out_arrs[0]).reshape(NCORES, L, D)
        out = np.zeros((B, L, D), np.float64)
        for c in range(NCORES):
            out[c // CPB] += full[c]
        return out.astype(np.float32)

    return run, collect



# revision 23
# speedup vs baseline: 30.2172x; 1.1978x over previous
"""Multi-head causal attention (B=2, L=2048, D=1024, H=16, Hd=64) on 8 TRN2
NeuronCores.

Sharding: data-parallel over the 2 batches x tensor-parallel over heads
(4 cores per batch, 4 heads per core).  Each core computes its heads'
QKV projection, attention, and a partial out-projection over its 256
local dims; the host sums the 4 partials per batch.

Per-core dataflow (all matmuls float32r = full-rate fp32 storage):
  qT,kT  [512, L]  = wqkT.T @ xT          (scale 1/8 folded into wq rows)
  v      [L, 256]  = xT.T-tiles @ wvT     ([l,d] layout, 65-strided cols + ones)
  S^T    [128k, 512q] = kT_h.T @ qT_h     (K=64)
  E      = exp(S^T + causal/mask bias)    (no max-subtraction needed; scores O(1))
  [attnT_h; denom] [65, 512q] += [v_h|1].T @ E   (accumulated over k tiles)
  attnT  normalized via 1/denom (gpsimd partition_broadcast; its ucode reads
         the physical tile start, so the reciprocal lives in a base-0 tile)
  out    [L, 1024] += attnT-pair.T @ woT-pair    (K=128 per head pair)

Causality lets q-tile t's attention start right after QKV chunk t, so the
emission interleaves projection chunks with attention units; one shared
8-bank PSUM pool (qkps 1 + vps 1 + st 2x2 + av 2) serves all phases, with
the out-projection accumulating in the "st" banks (idle once attention is
done) so the next rep's projection — which needs qkps/vps — isn't
serialized behind it.

For steady-state timing, `_build(reps=R)` emits the full body R times in
one program: x^T is loaded as half-resident quarter tiles, and the
qkl/vtg result tiles (plus wv) are double-buffered, so rep r+1's input
DMA and QKV projection overlap rep r's attention tail.  Every rep is a
complete execution (input DMA -> compute -> output DMA of the full
result), so the program's final output equals a single execution's.
"""
import sys
sys.path.insert(0, '/opt/trn_rl_repo')
import numpy as np

B, L, D = 2, 2048, 1024
H, HD = 16, 64
NCORES = 8
CPB = 4              # cores per batch
HPC = H // CPB       # heads per core = 4
DLOC = HPC * HD      # 256 local head dims per core
NKT, NQT = L // 128, L // 512   # 16 k-tiles, 4 q-tiles
NEG = -30000.0

_built = {}


def _build(status, use_cb, reps=1, tweaks=frozenset()):
    """status: [NKT, NQT] int8 (0=skip, 1=full, 2=mixed); use_cb: causal
    on-chip bias patterns (True) vs DMA'd bias tiles (False).

    reps: emit the full body (input DMA -> QKV -> attention -> out-proj ->
    output DMA) that many times in one program.  Tile tags are shared
    across reps, so buffers are reused and the framework serializes reps
    through WAR/RAW edges while still overlapping rep r+1's input DMA with
    rep r's compute tail.  Every rep recomputes the identical full result,
    so the final output equals a single execution's output; timing R reps
    in one launch amortizes the per-launch dispatch cost when measuring
    steady-state per-execution time."""
    import concourse.mybir as mybir
    import concourse.tile as tile
    from concourse import bacc

    F32 = mybir.dt.float32
    F32R = mybir.dt.float32r
    Exp = mybir.ActivationFunctionType.Exp

    # mixed-block index map for the DMA'd-bias mode
    mixed_ids = {}
    for qt in range(NQT):
        for kt in range(NKT):
            if status[kt, qt] == 2:
                mixed_ids[(kt, qt)] = len(mixed_ids)
    nmix = len(mixed_ids)

    nc = bacc.Bacc("TRN2", target_bir_lowering=False, debug=False)
    xT_d = nc.dram_tensor("xT", [D, L], F32R, kind="ExternalInput")
    wqkT_d = nc.dram_tensor("wqkT", [D, 2 * DLOC], F32R, kind="ExternalInput")
    wvT_d = nc.dram_tensor("wvT", [D, DLOC], F32R, kind="ExternalInput")
    woT_d = nc.dram_tensor("woT", [128, 2 * D], F32R, kind="ExternalInput")
    if not use_cb and nmix:
        bias_d = nc.dram_tensor("bias", [nmix, 128, 512], F32, kind="ExternalInput")
    out_d = nc.dram_tensor("out", [L, D], F32, kind="ExternalOutput")

    with tile.TileContext(nc) as tc:
        # One PSUM pool for every phase, per-tag budgets summing to the
        # 8 banks: qkps 1 + vps 1 + st 2x2 + av 2 = 8.  (A phase-scoped
        # pool would act as a barrier: attention banks couldn't allocate
        # until the QKV pool drained.)  Out-projection borrows the "st"
        # slots.  All pools stay open across reps so cross-rep overlap is
        # possible; shared tags serialize conflicting accesses.
        with tc.tile_pool(name="const", bufs=1) as const, \
             tc.tile_pool(name="esp", bufs=(3 if "esp3" in tweaks else 2)) as esp, \
             tc.tile_pool(name="misc", bufs=2) as misc, \
             tc.tile_pool(name="otp", bufs=2) as otp, \
             tc.tile_pool(name="psum", bufs=1, space="PSUM") as psum, \
             tc.tile_pool(name="atp", bufs=(3 if "atp3" in tweaks else 2)) as atp:
            for _rep in range(reps):
                _emit_rep(nc, tc, status, use_cb, mixed_ids,
                          const, esp, misc, otp, psum, atp,
                          xT_d, wqkT_d, wvT_d, woT_d,
                          bias_d if (not use_cb and nmix) else None, out_d,
                          tweaks)
    nc.compile()
    return nc


def _emit_rep(nc, tc, status, use_cb, mixed_ids,
              const, esp, misc, otp, psum, atp,
              xT_d, wqkT_d, wvT_d, woT_d, bias_d, out_d,
              tweaks=frozenset()):
    """tweaks: timing-attribution variants (experiments only, never used by
    the production kernel()/make_runner paths): "dveexp" replaces the exp
    activation with a DVE copy (wrong numerics, frees the ACT engine);
    "noattn"/"noout"/"noqkv" skip whole phases (wrong numerics)."""
    import concourse.mybir as mybir

    F32 = mybir.dt.float32
    F32R = mybir.dt.float32r
    Exp = mybir.ActivationFunctionType.Exp

    if True:
        if True:
            # ---- input loads (split across the SP and ACT HWDGE rings;
            # ordered so the first QKV groups aren't starved: wqk first,
            # then all x^T halves, weights wv/wo behind them) ----
            # wqk as 4 per-m-group tiles so the first projection group
            # only waits on 0.5 MB; issue order interleaves the weight
            # quarters with the first-half x^T tiles on both rings
            # x^T is loaded as 32 per-(k, l-chunk) quarter tiles [128, 512]
            # with tags shared between l-chunks lt and lt+2, so only half of
            # x is SBUF-resident at a time (x is only read by the QKV phase,
            # which consumes chunks in order; the freed 32 KB/partition pays
            # for the qkl/vtg double buffers that unlock cross-rep overlap).
            wqr = wqkT_d.ap().rearrange("(a p) m -> p a m", p=128)
            wqkg = [const.tile([128, D // 128, 128], F32R, tag=f"wqk{g}",
                               name=f"wqk{g}") for g in range(4)]
            xq = [[const.tile([128, 512], F32R, tag=f"xq{k}_{lt % 2}",
                              name=f"xq{k}_{lt}", bufs=1)
                   for lt in range(NQT)] for k in range(D // 128)]
            xr = xT_d.ap().rearrange("(a p) l -> a p l", p=128)
            # wv is read by AV matmuls until the very end of a rep: bufs=2
            # so the next rep's reload doesn't head-of-line block its ring
            # (and the PE queue behind the v matmuls waiting on it)
            wv = const.tile([128, D // 128, DLOC], F32R, tag="wv", bufs=2)
            wo = const.tile([128, 2 * D], F32R, tag="wo")
            nc.scalar.dma_start(out=wqkg[0],
                                in_=wqr[:, :, 0:128])
            for lt in range(NQT):
                for k in range(D // 128):
                    eng = nc.sync if k % 2 == 0 else nc.scalar
                    eng.dma_start(out=xq[k][lt],
                                  in_=xr[k][:, lt * 512:(lt + 1) * 512])
                    if lt == 0 and k == 1:
                        nc.scalar.dma_start(out=wqkg[1],
                                            in_=wqr[:, :, 128:256])
                if lt == 0:
                    nc.sync.dma_start(
                        out=wv,
                        in_=wvT_d.ap().rearrange("(a p) m -> p a m", p=128))
                    nc.scalar.dma_start(out=wqkg[2], in_=wqr[:, :, 256:384])
                    nc.sync.dma_start(out=wqkg[3], in_=wqr[:, :, 384:512])

            nc.scalar.dma_start(out=wo, in_=woT_d.ap())

            def xslice(l0, l1):
                lt = l0 // 512
                assert l1 <= (lt + 1) * 512
                o = lt * 512
                return lambda k: xq[k][lt][:, l0 - o:l1 - o]

            # ---- causal 0/1 mask pattern for the 128-wide diagonal strip
            # (only the r=0 lower-triangle is ever read) ----
            if use_cb:
                cb = const.tile([128, 1, 128], F32R, tag="cb")
                nc.vector.memset(cb.bitcast(F32), 1.0)
                # keep 1.0 where -k + q >= 0 (attend), else 0.0
                nc.gpsimd.affine_select(
                    out=cb.bitcast(F32)[:, 0, :],
                    in_=cb.bitcast(F32)[:, 0, :],
                    compare_op=mybir.AluOpType.is_ge, fill=0.0,
                    base=0, channel_multiplier=-1,
                    pattern=[[1, 128]])

            # ---- QKV projection ----
            # per-L-tile result tiles so attention for q-tile 0 can start
            # after 1/4 of the projection work; bufs=2 so rep r+1's
            # projection can fill fresh buffers while rep r's attention is
            # still reading the old ones (cross-rep pipelining)
            qkl = [const.tile([128, 4, 512], F32R, tag=f"qk{lt}",
                               name=f"qk{lt}", bufs=2)
                   for lt in range(NQT)]
            vtg = [const.tile([128, 4, HPC * (HD + 1)], F32R, tag=f"vt{g}",
                              name=f"vt{g}", bufs=2)
                   for g in range(NQT)]
            for g in range(NQT):
                # fill with 1.0; the v copies below overwrite all but the
                # per-head ones-columns (walrus rejects strided memsets)
                nc.vector.memset(vtg[g].bitcast(F32), 1.0)
            if True:

                def qkv_chunk(lt):
                    cp = nc.vector.tensor_copy
                    for g in range(4):     # interleave qk / v groups
                        ps = psum.tile([128, 512], F32, tag="qkps", bufs=1,
                                       name=f"qkps{lt}{g}")
                        xs = xslice(lt * 512, (lt + 1) * 512)
                        for kt in range(D // 128):
                            nc.tensor.matmul(
                                ps, wqkg[g][:, kt, :],
                                xs(kt),
                                start=(kt == 0), stop=(kt == D // 128 - 1))
                        cp(qkl[lt][:, g, :], ps)
                        l16 = 4 * lt + g
                        psv = psum.tile([128, DLOC], F32, tag="vps", bufs=1,
                                        name=f"vps{l16}")
                        xs = xslice(l16 * 128, (l16 + 1) * 128)
                        for kt in range(D // 128):
                            nc.tensor.matmul(
                                psv, xs(kt), wv[:, kt, :],
                                start=(kt == 0), stop=(kt == D // 128 - 1))
                        cp(vtg[lt][:, g, :]
                           .rearrange("p (h c) -> p h c", c=HD + 1)[:, :, 0:HD],
                           psv.rearrange("p (h c) -> p h c", c=HD))

                # ---- attention for one q-tile ----
                # Heads are processed in pairs (2hp, 2hp+1) living at
                # partition bases 0 / 64 of m-tile hp: their K=64 S^T matmuls
                # target disjoint PE row groups and run concurrently; exp is
                # fused over the pair ([128, 2, 512] per ACT op).
                at_tiles = {}

                def attention_unit(qt, hp):
                    # one attnT tile per head pair so the out-projection's
                    # p=0 matmuls can start while pair 1 still normalizes
                    if qt not in at_tiles:
                        at_tiles[qt] = [
                            atp.tile([128, 512], F32R, tag=f"at{p}",
                                     name=f"at{p}_{qt}") for p in range(2)]
                    ats = at_tiles[qt]
                    kts = [kt for kt in range(NKT) if status[kt, qt] != 0]
                    if True:
                        he, ho = 2 * hp, 2 * hp + 1
                        mq, mk = hp, 2 + hp
                        av = psum.tile([65, 2, 512], F32, tag="av", bufs=1,
                                       name=f"av{qt}{hp}")
                        for i, kt in enumerate(kts):
                            # causal mixed block at offset r: q-columns
                            # < 128r never attend this k-tile — shrink every
                            # op to the valid strip [c0:512] (the first kt of
                            # each q-tile is always full width, so the av
                            # accumulation bank is fully initialized)
                            mixed = status[kt, qt] == 2
                            c0 = 128 * (kt - 4 * qt) if (mixed and use_cb) \
                                else 0
                            st = psum.tile([128, 2, 512], F32, tag="st",
                                           bufs=2, name=f"st{qt}{hp}{kt}")
                            for j, base in ((0, 0), (1, 64)):
                                nc.tensor.matmul(
                                    st[:, j, c0:],
                                    qkl[kt // 4][base:base + 64, mk,
                                                 (kt % 4) * 128:
                                                 (kt % 4 + 1) * 128],
                                    qkl[qt][base:base + 64, mq, c0:],
                                    start=True, stop=True)
                            if mixed and not use_cb:
                                b_ap = misc.tile([128, 512], F32, tag="bt")
                                nc.sync.dma_start(
                                    out=b_ap,
                                    in_=bias_d.ap()[mixed_ids[(kt, qt)]])
                                for j in range(2):
                                    nc.vector.tensor_add(
                                        st[:, j, :], st[:, j, :], b_ap)
                            es = esp.tile([128, 2, 512], F32R, tag="es")
                            if "dveexp" in tweaks:
                                nc.vector.tensor_copy(
                                    es[:, :, c0:], st[:, :, c0:])
                            else:
                                nc.scalar.activation(es[:, :, c0:],
                                                     st[:, :, c0:], Exp)
                            if mixed and use_cb:
                                # only the 128-wide diagonal strip
                                # [c0, c0+128) is partial; it follows the
                                # r=0 triangle.  Columns < c0 are never read
                                # (every op above starts at c0), columns
                                # >= c0+128 attend fully.
                                nc.vector.tensor_mul(
                                    es[:, :, c0:c0 + 128],
                                    es[:, :, c0:c0 + 128],
                                    cb[:, 0:1, 0:128].broadcast_to(
                                        [128, 2, 128]))
                            for j, h in ((0, he), (1, ho)):
                                nc.tensor.matmul(
                                    av[:, j, c0:],
                                    vtg[kt // 4][:, kt % 4,
                                                 h * (HD + 1):(h + 1) * (HD + 1)],
                                    es[:, j, c0:],
                                    start=(i == 0), stop=(i == len(kts) - 1),
                                    skip_group_check=True)
                        # Free the av bank with one copy; normalize from the
                        # SBUF snapshot off the PE-critical path:
                        # attnT_h = av[0:64] / av[64]
                        # reciprocal must not be in-place (DVE in==out
                        # aliasing breaks on HW) and partition_broadcast's
                        # source must sit at partition 0 (the ucode reads
                        # physical partition 0, ignoring the AP offset)
                        avs = misc.tile([65, 2, 512], F32, tag="avs",
                                        bufs=1)
                        nc.vector.tensor_copy(avs, av)
                        for j, base in ((0, 0), (1, 64)):
                            # pbcast's ucode reads from the physical tile
                            # start: give each j its own base-0 source tile
                            rc = misc.tile([1, 512], F32, tag=f"rc{j}",
                                           name=f"rc{j}", bufs=1)
                            nc.vector.reciprocal(rc, avs[64:65, j, :])
                            bc = misc.tile([64, 512], F32, tag="bc",
                                           bufs=1)
                            nc.gpsimd.partition_broadcast(bc, rc, channels=64)
                            nc.vector.tensor_mul(
                                ats[hp][base:base + 64, :],
                                avs[0:64, j, :], bc)

                def outproj_chunk(qt):
                    # out-projection for this q-tile.  Accumulates in an
                    # "st"-tagged PSUM pair (idle once this rep's attention
                    # is done) rather than qkps/vps, so the NEXT rep's QKV
                    # projection — which needs qkps/vps — isn't serialized
                    # behind this rep's final out-projection.
                    if qt not in at_tiles:   # "noattn" tweak: garbage attnT
                        at_tiles[qt] = [
                            atp.tile([128, 512], F32R, tag=f"at{p}",
                                     name=f"at{p}_{qt}") for p in range(2)]
                    ats = at_tiles[qt]
                    for lt in range(4):
                        row = qt * 512 + lt * 128
                        po = psum.tile([128, 2, 512], F32, tag="st", bufs=2,
                                       name=f"po{qt}{lt}")
                        ot = otp.tile([128, 2, 512], F32, tag="ot")
                        for do in range(2):
                            for p in range(2):
                                nc.tensor.matmul(
                                    po[:, do, :],
                                    ats[p][:, lt * 128:(lt + 1) * 128],
                                    wo[:, p * D + do * 512:p * D + do * 512 + 512],
                                    start=(p == 0), stop=(p == 1),
                                    skip_group_check=True)
                        nc.vector.tensor_copy(ot, po)
                        oeng = (nc.sync if ("dmabal" not in tweaks
                                            or (qt * 4 + lt) % 2 == 0)
                                else nc.scalar)
                        oeng.dma_start(
                            out=out_d.ap()[row:row + 128, :],
                            in_=ot.rearrange("p a b -> p (a b)"))

                if "noqkv" in tweaks:
                    qkv_chunk = lambda lt: None
                if "noattn" in tweaks:
                    attention_unit = lambda qt, hp: None
                if "noout" in tweaks:
                    outproj_chunk = lambda qt: None

                if use_cb:
                    # causal: q-tile qt only needs qkl/vtg up to chunk qt —
                    # stagger so exp/attention overlap the projection, and
                    # interleave the qt=2/3 units so each pair's normalize
                    # latency hides under the other's matmuls
                    qkv_chunk(0)
                    qkv_chunk(1)
                    attention_unit(0, 0)
                    attention_unit(0, 1)
                    outproj_chunk(0)
                    qkv_chunk(2)
                    attention_unit(1, 0)
                    attention_unit(1, 1)
                    outproj_chunk(1)
                    qkv_chunk(3)
                    attention_unit(2, 0)
                    attention_unit(3, 0)
                    attention_unit(2, 1)
                    attention_unit(3, 1)
                    outproj_chunk(2)
                    outproj_chunk(3)
                else:
                    for lt in range(NQT):
                        qkv_chunk(lt)
                    for qt in range(NQT):
                        attention_unit(qt, 0)
                        attention_unit(qt, 1)
                        outproj_chunk(qt)


def _host_prep(x, mask, w_qkv, w_out):
    x = np.ascontiguousarray(np.asarray(x, dtype=np.float32))
    mask = np.asarray(mask).astype(bool)
    w_qkv = np.asarray(w_qkv, dtype=np.float32)
    w_out = np.asarray(w_out, dtype=np.float32)

    tril = np.tril(np.ones((L, L), dtype=bool))
    is_causal = all(np.array_equal(mask[b], tril) for b in range(B))

    # block classification on the S^T layout: block (kt, qt) covers
    # k in [kt*128, ...), q in [qt*512, ...)
    status = np.zeros((NKT, NQT), np.int8)
    if is_causal:
        for qt in range(NQT):
            for kt in range(NKT):
                r = kt - 4 * qt
                status[kt, qt] = 0 if r > 3 else (2 if r >= 0 else 1)
    else:
        for qt in range(NQT):
            for kt in range(NKT):
                blk = mask[:, qt * 512:(qt + 1) * 512, kt * 128:(kt + 1) * 128]
                status[kt, qt] = 1 if blk.all() else (0 if not blk.any() else 2)

    # per-core inputs
    scale = float(HD) ** -0.5
    in_maps = []
    bias_by_batch = None
    if not is_causal:
        mixed = [(kt, qt) for qt in range(NQT) for kt in range(NKT)
                 if status[kt, qt] == 2]
        if mixed:
            bias_by_batch = []
            for b in range(B):
                tiles = np.zeros((len(mixed), 128, 512), np.float32)
                mt = mask[b].T  # [k, q]
                for i, (kt, qt) in enumerate(mixed):
                    blk = mt[kt * 128:(kt + 1) * 128, qt * 512:(qt + 1) * 512]
                    tiles[i] = np.where(blk, 0.0, NEG)
                bias_by_batch.append(tiles)

    for c in range(NCORES):
        b = c // CPB
        hq = (c % CPB) * HPC
        wq = w_qkv[hq * HD:(hq + HPC) * HD] * scale
        wk = w_qkv[D + hq * HD:D + (hq + HPC) * HD]
        wv = w_qkv[2 * D + hq * HD:2 * D + (hq + HPC) * HD]
        wqkT = np.ascontiguousarray(np.concatenate([wq, wk], 0).T)
        wvT = np.ascontiguousarray(wv.T)
        wo_loc = w_out[:, hq * HD:(hq + HPC) * HD].T       # [256, 1024]
        woT = np.ascontiguousarray(
            wo_loc.reshape(2, 128, D).transpose(1, 0, 2).reshape(128, 2 * D))
        im = {"xT": np.ascontiguousarray(x[b].T), "wqkT": wqkT,
              "wvT": wvT, "woT": woT}
        if bias_by_batch is not None:
            im["bias"] = bias_by_batch[b]
        in_maps.append(im)
    return status, is_causal, in_maps


LAST_RESULTS = None


def kernel(x, mask, w_qkv, w_out):
    from concourse.bass_utils import run_bass_kernel_spmd
    global LAST_RESULTS

    status, is_causal, in_maps = _host_prep(x, mask, w_qkv, w_out)
    key = (is_causal, status.tobytes(), 1)
    if key not in _built:
        _built[key] = _build(status, is_causal)
    nc = _built[key]

    res = run_bass_kernel_spmd(nc, in_maps, core_ids=list(range(NCORES)))
    LAST_RESULTS = res
    out = np.zeros((B, L, D), np.float64)
    for c in range(NCORES):
        out[c // CPB] += res.results[c]["out"].astype(np.float64)
    return out.astype(np.float32)


def make_runner(x, mask, w_qkv, w_out, reps=1):
    """Persistent jitted runner over 8 cores with device-resident inputs,
    for steady-state timing (mirrors bass2jax.run_bass_via_pjrt without
    output donation — this kernel writes every output element).

    reps>1 builds a program with the full kernel body repeated that many
    times back-to-back on device (each rep re-loads inputs from HBM and
    re-writes the full output), so one launch measures `reps` executions
    and the per-execution time isn't swamped by per-launch dispatch
    overhead.  The returned output is the last rep's (identical) result."""
    import jax
    import numpy as jnp_np
    from jax.sharding import Mesh, PartitionSpec, NamedSharding
    from jax.experimental.shard_map import shard_map
    from concourse import bass2jax
    import concourse.mybir as mybir

    bass2jax.install_neuronx_cc_hook()
    status, is_causal, in_maps = _host_prep(x, mask, w_qkv, w_out)
    key = (is_causal, status.tobytes(), reps)
    if key not in _built:
        _built[key] = _build(status, is_causal, reps=reps)
    nc = _built[key]

    partition_name = (nc.partition_id_tensor.name
                      if nc.partition_id_tensor else None)
    in_names, out_names, out_avals = [], [], []
    for alloc in nc.m.functions[0].allocations:
        if not isinstance(alloc, mybir.MemoryLocationSet):
            continue
        name = alloc.memorylocations[0].name
        if alloc.kind == "ExternalInput":
            if name != partition_name:
                in_names.append(name)
        elif alloc.kind == "ExternalOutput":
            out_names.append(name)
            out_avals.append(jax.core.ShapedArray(
                tuple(alloc.tensor_shape), mybir.dt.np(alloc.dtype)))
    n_params = len(in_names)
    all_in_names = in_names + out_names
    if partition_name is not None:
        all_in_names.append(partition_name)

    def _body(*args):
        operands = list(args)
        if partition_name is not None:
            operands.append(bass2jax.partition_id_tensor())
        outs = bass2jax._bass_exec_p.bind(
            *operands, out_avals=tuple(out_avals), in_names=tuple(all_in_names),
            out_names=tuple(out_names), lowering_input_output_aliases=(),
            sim_require_finite=True, sim_require_nnan=True, nc=nc)
        return tuple(outs)

    devices = jax.devices()[:NCORES]
    mesh = Mesh(np.asarray(devices), ("core",))
    spec = NamedSharding(mesh, PartitionSpec("core"))
    sharded = jax.jit(
        shard_map(_body, mesh=mesh,
                  in_specs=(PartitionSpec("core"),) * (n_params + len(out_names)),
                  out_specs=(PartitionSpec("core"),) * len(out_names),
                  check_rep=False),
        keep_unused=True)
    concat_in = [
        jax.device_put(
            np.concatenate([in_maps[c][n] for c in range(NCORES)], 0), spec)
        for n in in_names]
    concat_zeros = [
        jax.device_put(
            np.zeros((NCORES * a.shape[0], *a.shape[1:]), a.dtype), spec)
        for a in out_avals]

    def run():
        return sharded(*concat_in, *concat_zeros)

    def collect(out_arrs):
        full = np.asarray(out_arrs[0]).reshape(NCORES, L, D)
        out = np.zeros((B, L, D), np.float64)
        for c in range(NCORES):
            out[c // CPB] += full[c]
        return out.astype(np.float32)

    return run, collect



# revision 25
# speedup vs baseline: 32.9822x; 1.0915x over previous
"""Multi-head causal attention (B=2, L=2048, D=1024, H=16, Hd=64) on 8 TRN2
NeuronCores.

Sharding: data-parallel over the 2 batches x tensor-parallel over heads
(4 cores per batch, 4 heads per core).  Each core computes its heads'
QKV projection, attention, and a partial out-projection over its 256
local dims; the host sums the 4 partials per batch.

Per-core dataflow (all matmuls float32r = full-rate fp32 storage):
  qT,kT  [512, L]  = wqkT.T @ xT          (scale 1/8 folded into wq rows)
  v      [L, 256]  = xT.T-tiles @ wvT     ([l,d] layout, 65-strided cols + ones)
  S^T    [128k, 512q] = kT_h.T @ qT_h     (K=64)
  E      = exp(S^T + causal/mask bias)    (no max-subtraction needed; scores O(1))
  [attnT_h; denom] [65, 512q] += [v_h|1].T @ E   (accumulated over k tiles)
  attnT  normalized via 1/denom (gpsimd partition_broadcast; its ucode reads
         the physical tile start, so the reciprocal lives in a base-0 tile)
  out    [L, 1024] += attnT-pair.T @ woT-pair    (K=128 per head pair)

Causality lets q-tile t's attention start right after QKV chunk t, so the
emission interleaves projection chunks with attention units; one shared
8-bank PSUM pool (qkps 1 + vps 1 + st 2x2 + av 2) serves all phases, with
the out-projection accumulating in the "st" banks (idle once attention is
done) so the next rep's projection — which needs qkps/vps — isn't
serialized behind it.

For steady-state timing, `_build(reps=R)` emits the full body R times in
one program: x^T is loaded as half-resident quarter tiles, and the
qkl/vtg result tiles (plus wv) are double-buffered, so rep r+1's input
DMA and QKV projection overlap rep r's attention tail.  Every rep is a
complete execution (input DMA -> compute -> output DMA of the full
result), so the program's final output equals a single execution's.
"""
import sys
sys.path.insert(0, '/opt/trn_rl_repo')
import numpy as np

B, L, D = 2, 2048, 1024
H, HD = 16, 64
NCORES = 8
CPB = 4              # cores per batch
HPC = H // CPB       # heads per core = 4
DLOC = HPC * HD      # 256 local head dims per core
NKT, NQT = L // 128, L // 512   # 16 k-tiles, 4 q-tiles
NEG = -30000.0

_built = {}


def _build(status, use_cb, reps=1, tweaks=frozenset()):
    """status: [NKT, NQT] int8 (0=skip, 1=full, 2=mixed); use_cb: causal
    on-chip bias patterns (True) vs DMA'd bias tiles (False).

    reps: emit the full body (input DMA -> QKV -> attention -> out-proj ->
    output DMA) that many times in one program.  Tile tags are shared
    across reps, so buffers are reused and the framework serializes reps
    through WAR/RAW edges while still overlapping rep r+1's input DMA with
    rep r's compute tail.  Every rep recomputes the identical full result,
    so the final output equals a single execution's output; timing R reps
    in one launch amortizes the per-launch dispatch cost when measuring
    steady-state per-execution time."""
    import concourse.mybir as mybir
    import concourse.tile as tile
    from concourse import bacc

    F32 = mybir.dt.float32
    F32R = mybir.dt.float32r
    Exp = mybir.ActivationFunctionType.Exp

    # mixed-block index map for the DMA'd-bias mode
    mixed_ids = {}
    for qt in range(NQT):
        for kt in range(NKT):
            if status[kt, qt] == 2:
                mixed_ids[(kt, qt)] = len(mixed_ids)
    nmix = len(mixed_ids)

    nc = bacc.Bacc("TRN2", target_bir_lowering=False, debug=False)
    xT_d = nc.dram_tensor("xT", [D, L], F32R, kind="ExternalInput")
    wqkT_d = nc.dram_tensor("wqkT", [D, 2 * DLOC], F32R, kind="ExternalInput")
    wvT_d = nc.dram_tensor("wvT", [D, DLOC], F32R, kind="ExternalInput")
    woT_d = nc.dram_tensor("woT", [128, 2 * D], F32R, kind="ExternalInput")
    if not use_cb and nmix:
        bias_d = nc.dram_tensor("bias", [nmix, 128, 512], F32, kind="ExternalInput")
    out_d = nc.dram_tensor("out", [L, D], F32, kind="ExternalOutput")

    with tile.TileContext(nc) as tc:
        # One PSUM pool for every phase, per-tag budgets summing to the
        # 8 banks: qkps 1 + vps 1 + st 2x2 + av 2 = 8.  (A phase-scoped
        # pool would act as a barrier: attention banks couldn't allocate
        # until the QKV pool drained.)  Out-projection borrows the "st"
        # slots.  All pools stay open across reps so cross-rep overlap is
        # possible; shared tags serialize conflicting accesses.
        with tc.tile_pool(name="const", bufs=1) as const, \
             tc.tile_pool(name="esp", bufs=(3 if "esp3" in tweaks else 2)) as esp, \
             tc.tile_pool(name="misc", bufs=2) as misc, \
             tc.tile_pool(name="otp", bufs=2) as otp, \
             tc.tile_pool(name="psum", bufs=1, space="PSUM") as psum, \
             tc.tile_pool(name="atp", bufs=(3 if "atp3" in tweaks else 2)) as atp:
            for _rep in range(reps):
                _emit_rep(nc, tc, status, use_cb, mixed_ids,
                          const, esp, misc, otp, psum, atp,
                          xT_d, wqkT_d, wvT_d, woT_d,
                          bias_d if (not use_cb and nmix) else None, out_d,
                          tweaks)
    nc.compile()
    return nc


def _emit_rep(nc, tc, status, use_cb, mixed_ids,
              const, esp, misc, otp, psum, atp,
              xT_d, wqkT_d, wvT_d, woT_d, bias_d, out_d,
              tweaks=frozenset()):
    """tweaks: timing-attribution variants (experiments only, never used by
    the production kernel()/make_runner paths): "dveexp" replaces the exp
    activation with a DVE copy (wrong numerics, frees the ACT engine);
    "noattn"/"noout"/"noqkv" skip whole phases (wrong numerics)."""
    import concourse.mybir as mybir

    F32 = mybir.dt.float32
    F32R = mybir.dt.float32r
    Exp = mybir.ActivationFunctionType.Exp

    if True:
        if True:
            # ---- input loads (split across the SP and ACT HWDGE rings;
            # ordered so the first QKV groups aren't starved: wqk first,
            # then all x^T halves, weights wv/wo behind them) ----
            # wqk as 4 per-m-group tiles so the first projection group
            # only waits on 0.5 MB; issue order interleaves the weight
            # quarters with the first-half x^T tiles on both rings
            # x^T is loaded as 32 per-(k, l-chunk) quarter tiles [128, 512]
            # with tags shared between l-chunks lt and lt+2, so only half of
            # x is SBUF-resident at a time (x is only read by the QKV phase,
            # which consumes chunks in order; the freed 32 KB/partition pays
            # for the qkl/vtg double buffers that unlock cross-rep overlap).
            wqr = wqkT_d.ap().rearrange("(a p) m -> p a m", p=128)
            wqkg = [const.tile([128, D // 128, 128], F32R, tag=f"wqk{g}",
                               name=f"wqk{g}") for g in range(4)]
            xq = [[const.tile([128, 512], F32R, tag=f"xq{k}_{lt % 2}",
                              name=f"xq{k}_{lt}", bufs=1)
                   for lt in range(NQT)] for k in range(D // 128)]
            xr = xT_d.ap().rearrange("(a p) l -> a p l", p=128)
            # wv is read by AV matmuls until the very end of a rep: bufs=2
            # so the next rep's reload doesn't head-of-line block its ring
            # (and the PE queue behind the v matmuls waiting on it)
            wv = const.tile([128, D // 128, DLOC], F32R, tag="wv", bufs=2)
            wo = const.tile([128, 2 * D], F32R, tag="wo")
            nc.scalar.dma_start(out=wqkg[0],
                                in_=wqr[:, :, 0:128])
            for lt in range(NQT):
                for k in range(D // 128):
                    eng = nc.sync if k % 2 == 0 else nc.scalar
                    eng.dma_start(out=xq[k][lt],
                                  in_=xr[k][:, lt * 512:(lt + 1) * 512])
                    if lt == 0 and k == 1:
                        nc.scalar.dma_start(out=wqkg[1],
                                            in_=wqr[:, :, 128:256])
                if lt == 0:
                    nc.sync.dma_start(
                        out=wv,
                        in_=wvT_d.ap().rearrange("(a p) m -> p a m", p=128))
                    nc.scalar.dma_start(out=wqkg[2], in_=wqr[:, :, 256:384])
                    nc.sync.dma_start(out=wqkg[3], in_=wqr[:, :, 384:512])

            nc.scalar.dma_start(out=wo, in_=woT_d.ap())

            def xslice(l0, l1):
                lt = l0 // 512
                assert l1 <= (lt + 1) * 512
                o = lt * 512
                return lambda k: xq[k][lt][:, l0 - o:l1 - o]

            # ---- causal 0/1 mask pattern for the 128-wide diagonal strip
            # (only the r=0 lower-triangle is ever read) ----
            if use_cb:
                cb = const.tile([128, 1, 128], F32R, tag="cb")
                nc.vector.memset(cb.bitcast(F32), 1.0)
                # keep 1.0 where -k + q >= 0 (attend), else 0.0
                nc.gpsimd.affine_select(
                    out=cb.bitcast(F32)[:, 0, :],
                    in_=cb.bitcast(F32)[:, 0, :],
                    compare_op=mybir.AluOpType.is_ge, fill=0.0,
                    base=0, channel_multiplier=-1,
                    pattern=[[1, 128]])

            # ---- QKV projection ----
            # per-L-tile result tiles so attention for q-tile 0 can start
            # after 1/4 of the projection work; bufs=2 so rep r+1's
            # projection can fill fresh buffers while rep r's attention is
            # still reading the old ones (cross-rep pipelining)
            qkl = [const.tile([128, 4, 512], F32R, tag=f"qk{lt}",
                               name=f"qk{lt}", bufs=2)
                   for lt in range(NQT)]
            vtg = [const.tile([128, 4, HPC * (HD + 1)], F32R, tag=f"vt{g}",
                              name=f"vt{g}", bufs=2)
                   for g in range(NQT)]
            for g in range(NQT):
                # fill with 1.0; the v copies below overwrite all but the
                # per-head ones-columns (walrus rejects strided memsets)
                nc.vector.memset(vtg[g].bitcast(F32), 1.0)
            if True:

                def qkv_chunk(lt):
                    cp = nc.vector.tensor_copy
                    for g in range(4):     # interleave qk / v groups
                        ps = psum.tile([128, 512], F32, tag="qkps", bufs=1,
                                       name=f"qkps{lt}{g}")
                        xs = xslice(lt * 512, (lt + 1) * 512)
                        for kt in range(D // 128):
                            nc.tensor.matmul(
                                ps, wqkg[g][:, kt, :],
                                xs(kt),
                                start=(kt == 0), stop=(kt == D // 128 - 1))
                        cp(qkl[lt][:, g, :], ps)
                        l16 = 4 * lt + g
                        psv = psum.tile([128, DLOC], F32, tag="vps", bufs=1,
                                        name=f"vps{l16}")
                        xs = xslice(l16 * 128, (l16 + 1) * 128)
                        for kt in range(D // 128):
                            nc.tensor.matmul(
                                psv, xs(kt), wv[:, kt, :],
                                start=(kt == 0), stop=(kt == D // 128 - 1))
                        cp(vtg[lt][:, g, :]
                           .rearrange("p (h c) -> p h c", c=HD + 1)[:, :, 0:HD],
                           psv.rearrange("p (h c) -> p h c", c=HD))

                # ---- attention for one q-tile ----
                # Heads are processed in pairs (2hp, 2hp+1) living at
                # partition bases 0 / 64 of m-tile hp: their K=64 S^T matmuls
                # target disjoint PE row groups and run concurrently; exp is
                # fused over the pair ([128, 2, 512] per ACT op).
                at_tiles = {}

                def attention_unit(qt, hp):
                    # one attnT tile per head pair so the out-projection's
                    # p=0 matmuls can start while pair 1 still normalizes
                    if qt not in at_tiles:
                        at_tiles[qt] = [
                            atp.tile([128, 512], F32R, tag=f"at{p}",
                                     name=f"at{p}_{qt}") for p in range(2)]
                    ats = at_tiles[qt]
                    kts = [kt for kt in range(NKT) if status[kt, qt] != 0]
                    if True:
                        he, ho = 2 * hp, 2 * hp + 1
                        mq, mk = hp, 2 + hp
                        av = psum.tile([65, 2, 512], F32, tag="av", bufs=1,
                                       name=f"av{qt}{hp}")
                        for i, kt in enumerate(kts):
                            # causal mixed block at offset r: q-columns
                            # < 128r never attend this k-tile — shrink every
                            # op to the valid strip [c0:512] (the first kt of
                            # each q-tile is always full width, so the av
                            # accumulation bank is fully initialized)
                            mixed = status[kt, qt] == 2
                            c0 = 128 * (kt - 4 * qt) if (mixed and use_cb) \
                                else 0
                            st = psum.tile([128, 2, 512], F32, tag="st",
                                           bufs=2, name=f"st{qt}{hp}{kt}")
                            for j, base in ((0, 0), (1, 64)):
                                nc.tensor.matmul(
                                    st[:, j, c0:],
                                    qkl[kt // 4][base:base + 64, mk,
                                                 (kt % 4) * 128:
                                                 (kt % 4 + 1) * 128],
                                    qkl[qt][base:base + 64, mq, c0:],
                                    start=True, stop=True)
                            if mixed and not use_cb:
                                b_ap = misc.tile([128, 512], F32, tag="bt")
                                nc.sync.dma_start(
                                    out=b_ap,
                                    in_=bias_d.ap()[mixed_ids[(kt, qt)]])
                                for j in range(2):
                                    nc.vector.tensor_add(
                                        st[:, j, :], st[:, j, :], b_ap)
                            es = esp.tile([128, 2, 512], F32R, tag="es")
                            if "dveexp" in tweaks:
                                nc.vector.tensor_copy(
                                    es[:, :, c0:], st[:, :, c0:])
                            else:
                                nc.scalar.activation(es[:, :, c0:],
                                                     st[:, :, c0:], Exp)
                            if mixed and use_cb:
                                # only the 128-wide diagonal strip
                                # [c0, c0+128) is partial; it follows the
                                # r=0 triangle.  Columns < c0 are never read
                                # (every op above starts at c0), columns
                                # >= c0+128 attend fully.
                                nc.vector.tensor_mul(
                                    es[:, :, c0:c0 + 128],
                                    es[:, :, c0:c0 + 128],
                                    cb[:, 0:1, 0:128].broadcast_to(
                                        [128, 2, 128]))
                            for j, h in ((0, he), (1, ho)):
                                nc.tensor.matmul(
                                    av[:, j, c0:],
                                    vtg[kt // 4][:, kt % 4,
                                                 h * (HD + 1):(h + 1) * (HD + 1)],
                                    es[:, j, c0:],
                                    start=(i == 0), stop=(i == len(kts) - 1),
                                    skip_group_check=True)
                        # Free the av bank with one copy; normalize from the
                        # SBUF snapshot off the PE-critical path:
                        # attnT_h = av[0:64] / av[64]
                        # reciprocal must not be in-place (DVE in==out
                        # aliasing breaks on HW) and partition_broadcast's
                        # source must sit at partition 0 (the ucode reads
                        # physical partition 0, ignoring the AP offset)
                        avs = misc.tile([65, 2, 512], F32, tag="avs",
                                        bufs=1)
                        nc.vector.tensor_copy(avs, av)
                        for j, base in ((0, 0), (1, 64)):
                            # pbcast's ucode reads from the physical tile
                            # start: give each j its own base-0 source tile
                            rc = misc.tile([1, 512], F32, tag=f"rc{j}",
                                           name=f"rc{j}", bufs=1)
                            nc.vector.reciprocal(rc, avs[64:65, j, :])
                            bc = misc.tile([64, 512], F32, tag="bc",
                                           bufs=1)
                            nc.gpsimd.partition_broadcast(bc, rc, channels=64)
                            nc.vector.tensor_mul(
                                ats[hp][base:base + 64, :],
                                avs[0:64, j, :], bc)

                def outproj_chunk(qt):
                    # out-projection for this q-tile.  Accumulates in an
                    # "st"-tagged PSUM pair (idle once this rep's attention
                    # is done) rather than qkps/vps, so the NEXT rep's QKV
                    # projection — which needs qkps/vps — isn't serialized
                    # behind this rep's final out-projection.
                    if qt not in at_tiles:   # "noattn" tweak: garbage attnT
                        at_tiles[qt] = [
                            atp.tile([128, 512], F32R, tag=f"at{p}",
                                     name=f"at{p}_{qt}") for p in range(2)]
                    ats = at_tiles[qt]
                    for lt in range(4):
                        row = qt * 512 + lt * 128
                        ot = otp.tile([128, 2, 512], F32, tag="ot")
                        if "opqk" in tweaks:
                            # original placement: borrow the projection's
                            # qkps/vps banks (A/B variant)
                            pos = [psum.tile([128, 512], F32, tag=t, bufs=1,
                                             name=f"po{qt}{lt}{t}")
                                   for t in ("qkps", "vps")]
                            for do in range(2):
                                for p in range(2):
                                    nc.tensor.matmul(
                                        pos[do],
                                        ats[p][:, lt * 128:(lt + 1) * 128],
                                        wo[:, p * D + do * 512:
                                           p * D + do * 512 + 512],
                                        start=(p == 0), stop=(p == 1))
                                nc.vector.tensor_copy(ot[:, do, :], pos[do])
                        else:
                            po = psum.tile([128, 2, 512], F32, tag="st",
                                           bufs=2, name=f"po{qt}{lt}")
                            for do in range(2):
                                for p in range(2):
                                    nc.tensor.matmul(
                                        po[:, do, :],
                                        ats[p][:, lt * 128:(lt + 1) * 128],
                                        wo[:, p * D + do * 512:
                                           p * D + do * 512 + 512],
                                        start=(p == 0), stop=(p == 1),
                                        skip_group_check=True)
                            nc.vector.tensor_copy(ot, po)
                        oeng = (nc.sync if ("dmabal" not in tweaks
                                            or (qt * 4 + lt) % 2 == 0)
                                else nc.scalar)
                        oeng.dma_start(
                            out=out_d.ap()[row:row + 128, :],
                            in_=ot.rearrange("p a b -> p (a b)"))

                if "noqkv" in tweaks:
                    qkv_chunk = lambda lt: None
                if "noattn" in tweaks:
                    attention_unit = lambda qt, hp: None
                if "noout" in tweaks:
                    outproj_chunk = lambda qt: None

                if use_cb:
                    # causal: q-tile qt only needs qkl/vtg up to chunk qt —
                    # stagger so exp/attention overlap the projection, and
                    # interleave the qt=2/3 units so each pair's normalize
                    # latency hides under the other's matmuls
                    qkv_chunk(0)
                    qkv_chunk(1)
                    attention_unit(0, 0)
                    attention_unit(0, 1)
                    outproj_chunk(0)
                    qkv_chunk(2)
                    attention_unit(1, 0)
                    attention_unit(1, 1)
                    outproj_chunk(1)
                    qkv_chunk(3)
                    attention_unit(2, 0)
                    attention_unit(3, 0)
                    attention_unit(2, 1)
                    attention_unit(3, 1)
                    outproj_chunk(2)
                    outproj_chunk(3)
                else:
                    for lt in range(NQT):
                        qkv_chunk(lt)
                    for qt in range(NQT):
                        attention_unit(qt, 0)
                        attention_unit(qt, 1)
                        outproj_chunk(qt)


def _host_prep(x, mask, w_qkv, w_out):
    x = np.ascontiguousarray(np.asarray(x, dtype=np.float32))
    mask = np.asarray(mask).astype(bool)
    w_qkv = np.asarray(w_qkv, dtype=np.float32)
    w_out = np.asarray(w_out, dtype=np.float32)

    tril = np.tril(np.ones((L, L), dtype=bool))
    is_causal = all(np.array_equal(mask[b], tril) for b in range(B))

    # block classification on the S^T layout: block (kt, qt) covers
    # k in [kt*128, ...), q in [qt*512, ...)
    status = np.zeros((NKT, NQT), np.int8)
    if is_causal:
        for qt in range(NQT):
            for kt in range(NKT):
                r = kt - 4 * qt
                status[kt, qt] = 0 if r > 3 else (2 if r >= 0 else 1)
    else:
        for qt in range(NQT):
            for kt in range(NKT):
                blk = mask[:, qt * 512:(qt + 1) * 512, kt * 128:(kt + 1) * 128]
                status[kt, qt] = 1 if blk.all() else (0 if not blk.any() else 2)

    # per-core inputs
    scale = float(HD) ** -0.5
    in_maps = []
    bias_by_batch = None
    if not is_causal:
        mixed = [(kt, qt) for qt in range(NQT) for kt in range(NKT)
                 if status[kt, qt] == 2]
        if mixed:
            bias_by_batch = []
            for b in range(B):
                tiles = np.zeros((len(mixed), 128, 512), np.float32)
                mt = mask[b].T  # [k, q]
                for i, (kt, qt) in enumerate(mixed):
                    blk = mt[kt * 128:(kt + 1) * 128, qt * 512:(qt + 1) * 512]
                    tiles[i] = np.where(blk, 0.0, NEG)
                bias_by_batch.append(tiles)

    for c in range(NCORES):
        b = c // CPB
        hq = (c % CPB) * HPC
        wq = w_qkv[hq * HD:(hq + HPC) * HD] * scale
        wk = w_qkv[D + hq * HD:D + (hq + HPC) * HD]
        wv = w_qkv[2 * D + hq * HD:2 * D + (hq + HPC) * HD]
        wqkT = np.ascontiguousarray(np.concatenate([wq, wk], 0).T)
        wvT = np.ascontiguousarray(wv.T)
        wo_loc = w_out[:, hq * HD:(hq + HPC) * HD].T       # [256, 1024]
        woT = np.ascontiguousarray(
            wo_loc.reshape(2, 128, D).transpose(1, 0, 2).reshape(128, 2 * D))
        im = {"xT": np.ascontiguousarray(x[b].T), "wqkT": wqkT,
              "wvT": wvT, "woT": woT}
        if bias_by_batch is not None:
            im["bias"] = bias_by_batch[b]
        in_maps.append(im)
    return status, is_causal, in_maps


LAST_RESULTS = None


def kernel(x, mask, w_qkv, w_out):
    from concourse.bass_utils import run_bass_kernel_spmd
    global LAST_RESULTS

    status, is_causal, in_maps = _host_prep(x, mask, w_qkv, w_out)
    key = (is_causal, status.tobytes(), 1)
    if key not in _built:
        _built[key] = _build(status, is_causal)
    nc = _built[key]

    res = run_bass_kernel_spmd(nc, in_maps, core_ids=list(range(NCORES)))
    LAST_RESULTS = res
    out = np.zeros((B, L, D), np.float64)
    for c in range(NCORES):
        out[c // CPB] += res.results[c]["out"].astype(np.float64)
    return out.astype(np.float32)


def make_runner(x, mask, w_qkv, w_out, reps=1):
    """Persistent jitted runner over 8 cores with device-resident inputs,
    for steady-state timing (mirrors bass2jax.run_bass_via_pjrt without
    output donation — this kernel writes every output element).

    reps>1 builds a program with the full kernel body repeated that many
    times back-to-back on device (each rep re-loads inputs from HBM and
    re-writes the full output), so one launch measures `reps` executions
    and the per-execution time isn't swamped by per-launch dispatch
    overhead.  The returned output is the last rep's (identical) result."""
    import jax
    import numpy as jnp_np
    from jax.sharding import Mesh, PartitionSpec, NamedSharding
    from jax.experimental.shard_map import shard_map
    from concourse import bass2jax
    import concourse.mybir as mybir

    bass2jax.install_neuronx_cc_hook()
    status, is_causal, in_maps = _host_prep(x, mask, w_qkv, w_out)
    key = (is_causal, status.tobytes(), reps)
    if key not in _built:
        _built[key] = _build(status, is_causal, reps=reps)
    nc = _built[key]

    partition_name = (nc.partition_id_tensor.name
                      if nc.partition_id_tensor else None)
    in_names, out_names, out_avals = [], [], []
    for alloc in nc.m.functions[0].allocations:
        if not isinstance(alloc, mybir.MemoryLocationSet):
            continue
        name = alloc.memorylocations[0].name
        if alloc.kind == "ExternalInput":
            if name != partition_name:
                in_names.append(name)
        elif alloc.kind == "ExternalOutput":
            out_names.append(name)
            out_avals.append(jax.core.ShapedArray(
                tuple(alloc.tensor_shape), mybir.dt.np(alloc.dtype)))
    n_params = len(in_names)
    all_in_names = in_names + out_names
    if partition_name is not None:
        all_in_names.append(partition_name)

    def _body(*args):
        operands = list(args)
        if partition_name is not None:
            operands.append(bass2jax.partition_id_tensor())
        outs = bass2jax._bass_exec_p.bind(
            *operands, out_avals=tuple(out_avals), in_names=tuple(all_in_names),
            out_names=tuple(out_names), lowering_input_output_aliases=(),
            sim_require_finite=True, sim_require_nnan=True, nc=nc)
        return tuple(outs)

    devices = jax.devices()[:NCORES]
    mesh = Mesh(np.asarray(devices), ("core",))
    spec = NamedSharding(mesh, PartitionSpec("core"))
    sharded = jax.jit(
        shard_map(_body, mesh=mesh,
                  in_specs=(PartitionSpec("core"),) * (n_params + len(out_names)),
                  out_specs=(PartitionSpec("core"),) * len(out_names),
                  check_rep=False),
        keep_unused=True)
    concat_in = [
        jax.device_put(
            np.concatenate([in_maps[c][n] for c in range(NCORES)], 0), spec)
        for n in in_names]
    concat_zeros = [
        jax.device_put(
            np.zeros((NCORES * a.shape[0], *a.shape[1:]), a.dtype), spec)
        for a in out_avals]

    def run():
        return sharded(*concat_in, *concat_zeros)

    def collect(out_arrs):
        full = np.asarray(out_arrs[0]).reshape(NCORES, L, D)
        out = np.zeros((B, L, D), np.float64)
        for c in range(NCORES):
            out[c // CPB] += full[c]
        return out.astype(np.float32)

    return run, collect



# revision 28
# speedup vs baseline: 33.7468x; 1.0232x over previous
"""Multi-head causal attention (B=2, L=2048, D=1024, H=16, Hd=64) on 8 TRN2
NeuronCores.

Sharding: data-parallel over the 2 batches x tensor-parallel over heads
(4 cores per batch, 4 heads per core).  Each core computes its heads'
QKV projection, attention, and a partial out-projection over its 256
local dims; the host sums the 4 partials per batch.

Per-core dataflow (all matmuls float32r = full-rate fp32 storage):
  qT,kT  [512, L]  = wqkT.T @ xT          (scale 1/8 folded into wq rows)
  v      [L, 256]  = xT.T-tiles @ wvT     ([l,d] layout, 65-strided cols + ones)
  S^T    [128k, 512q] = kT_h.T @ qT_h     (K=64)
  E      = exp(S^T + causal/mask bias)    (no max-subtraction needed; scores O(1))
  [attnT_h; denom] [65, 512q] += [v_h|1].T @ E   (accumulated over k tiles)
  attnT  normalized via 1/denom (gpsimd partition_broadcast; its ucode reads
         the physical tile start, so the reciprocal lives in a base-0 tile)
  out    [L, 1024] += attnT-pair.T @ woT-pair    (K=128 per head pair)

Causality lets q-tile t's attention start right after QKV chunk t, so the
emission interleaves projection chunks with attention units; one shared
8-bank PSUM pool (qkps 1 + vps 1 + st 2x2 + av 2) serves all phases, with
the out-projection accumulating in the "st" banks (idle once attention is
done) so the next rep's projection — which needs qkps/vps — isn't
serialized behind it.

For steady-state timing, `_build(reps=R)` emits the full body R times in
one program: x^T is loaded as half-resident quarter tiles, and the
qkl/vtg result tiles (plus wv) are double-buffered, so rep r+1's input
DMA and QKV projection overlap rep r's attention tail.  Every rep is a
complete execution (input DMA -> compute -> output DMA of the full
result), so the program's final output equals a single execution's.
"""
import sys
sys.path.insert(0, '/opt/trn_rl_repo')
import numpy as np

B, L, D = 2, 2048, 1024
H, HD = 16, 64
NCORES = 8
CPB = 4              # cores per batch
HPC = H // CPB       # heads per core = 4
DLOC = HPC * HD      # 256 local head dims per core
NKT, NQT = L // 128, L // 512   # 16 k-tiles, 4 q-tiles
NEG = -30000.0

_built = {}


def _build(status, use_cb, reps=1, tweaks=frozenset()):
    """status: [NKT, NQT] int8 (0=skip, 1=full, 2=mixed); use_cb: causal
    on-chip bias patterns (True) vs DMA'd bias tiles (False).

    reps: emit the full body (input DMA -> QKV -> attention -> out-proj ->
    output DMA) that many times in one program.  Tile tags are shared
    across reps, so buffers are reused and the framework serializes reps
    through WAR/RAW edges while still overlapping rep r+1's input DMA with
    rep r's compute tail.  Every rep recomputes the identical full result,
    so the final output equals a single execution's output; timing R reps
    in one launch amortizes the per-launch dispatch cost when measuring
    steady-state per-execution time."""
    import concourse.mybir as mybir
    import concourse.tile as tile
    from concourse import bacc

    F32 = mybir.dt.float32
    F32R = mybir.dt.float32r
    Exp = mybir.ActivationFunctionType.Exp

    # mixed-block index map for the DMA'd-bias mode
    mixed_ids = {}
    for qt in range(NQT):
        for kt in range(NKT):
            if status[kt, qt] == 2:
                mixed_ids[(kt, qt)] = len(mixed_ids)
    nmix = len(mixed_ids)

    nc = bacc.Bacc("TRN2", target_bir_lowering=False, debug=False)
    xT_d = nc.dram_tensor("xT", [D, L], F32R, kind="ExternalInput")
    wqkT_d = nc.dram_tensor("wqkT", [D, 2 * DLOC], F32R, kind="ExternalInput")
    wvT_d = nc.dram_tensor("wvT", [D, DLOC], F32R, kind="ExternalInput")
    woT_d = nc.dram_tensor("woT", [128, 2 * D], F32R, kind="ExternalInput")
    if not use_cb and nmix:
        bias_d = nc.dram_tensor("bias", [nmix, 128, 512], F32, kind="ExternalInput")
    out_d = nc.dram_tensor("out", [L, D], F32, kind="ExternalOutput")

    with tile.TileContext(nc) as tc:
        # One PSUM pool for every phase, per-tag budgets summing to the
        # 8 banks: qkps 1 + vps 1 + st 2x2 + av 2 = 8.  (A phase-scoped
        # pool would act as a barrier: attention banks couldn't allocate
        # until the QKV pool drained.)  Out-projection borrows the "st"
        # slots.  All pools stay open across reps so cross-rep overlap is
        # possible; shared tags serialize conflicting accesses.
        with tc.tile_pool(name="const", bufs=1) as const, \
             tc.tile_pool(name="esp", bufs=(3 if "esp3" in tweaks else 2)) as esp, \
             tc.tile_pool(name="misc", bufs=2) as misc, \
             tc.tile_pool(name="otp", bufs=2) as otp, \
             tc.tile_pool(name="psum", bufs=1, space="PSUM") as psum, \
             tc.tile_pool(name="atp", bufs=(3 if "atp3" in tweaks else 2)) as atp:
            for _rep in range(reps):
                _emit_rep(nc, tc, status, use_cb, mixed_ids,
                          const, esp, misc, otp, psum, atp,
                          xT_d, wqkT_d, wvT_d, woT_d,
                          bias_d if (not use_cb and nmix) else None, out_d,
                          tweaks)
    nc.compile()
    return nc


def _emit_rep(nc, tc, status, use_cb, mixed_ids,
              const, esp, misc, otp, psum, atp,
              xT_d, wqkT_d, wvT_d, woT_d, bias_d, out_d,
              tweaks=frozenset()):
    """tweaks: timing-attribution variants (experiments only, never used by
    the production kernel()/make_runner paths): "dveexp" replaces the exp
    activation with a DVE copy (wrong numerics, frees the ACT engine);
    "noattn"/"noout"/"noqkv" skip whole phases (wrong numerics)."""
    import concourse.mybir as mybir

    F32 = mybir.dt.float32
    F32R = mybir.dt.float32r
    Exp = mybir.ActivationFunctionType.Exp

    if True:
        if True:
            # ---- input loads (split across the SP and ACT HWDGE rings;
            # ordered so the first QKV groups aren't starved: wqk first,
            # then all x^T halves, weights wv/wo behind them) ----
            # wqk as 4 per-m-group tiles so the first projection group
            # only waits on 0.5 MB; issue order interleaves the weight
            # quarters with the first-half x^T tiles on both rings
            # x^T is loaded as 32 per-(k, l-chunk) quarter tiles [128, 512]
            # with tags shared between l-chunks lt and lt+2, so only half of
            # x is SBUF-resident at a time (x is only read by the QKV phase,
            # which consumes chunks in order; the freed 32 KB/partition pays
            # for the qkl/vtg double buffers that unlock cross-rep overlap).
            wqr = wqkT_d.ap().rearrange("(a p) m -> p a m", p=128)
            wqkg = [const.tile([128, D // 128, 128], F32R, tag=f"wqk{g}",
                               name=f"wqk{g}") for g in range(4)]
            xq = [[const.tile([128, 512], F32R, tag=f"xq{k}_{lt % 2}",
                              name=f"xq{k}_{lt}", bufs=1)
                   for lt in range(NQT)] for k in range(D // 128)]
            xr = xT_d.ap().rearrange("(a p) l -> a p l", p=128)
            # wv is read by AV matmuls until the very end of a rep: bufs=2
            # so the next rep's reload doesn't head-of-line block its ring
            # (and the PE queue behind the v matmuls waiting on it)
            wv = const.tile([128, D // 128, DLOC], F32R, tag="wv", bufs=2)
            wo = const.tile([128, 2 * D], F32R, tag="wo")
            nc.scalar.dma_start(out=wqkg[0],
                                in_=wqr[:, :, 0:128])
            for lt in range(NQT):
                for k in range(D // 128):
                    eng = nc.sync if k % 2 == 0 else nc.scalar
                    eng.dma_start(out=xq[k][lt],
                                  in_=xr[k][:, lt * 512:(lt + 1) * 512])
                    if lt == 0 and k == 1:
                        nc.scalar.dma_start(out=wqkg[1],
                                            in_=wqr[:, :, 128:256])
                if lt == 0:
                    nc.sync.dma_start(
                        out=wv,
                        in_=wvT_d.ap().rearrange("(a p) m -> p a m", p=128))
                    nc.scalar.dma_start(out=wqkg[2], in_=wqr[:, :, 256:384])
                    nc.sync.dma_start(out=wqkg[3], in_=wqr[:, :, 384:512])

            nc.scalar.dma_start(out=wo, in_=woT_d.ap())

            def xslice(l0, l1):
                lt = l0 // 512
                assert l1 <= (lt + 1) * 512
                o = lt * 512
                return lambda k: xq[k][lt][:, l0 - o:l1 - o]

            # ---- causal 0/1 mask pattern for the 128-wide diagonal strip
            # (only the r=0 lower-triangle is ever read) ----
            if use_cb:
                cb = const.tile([128, 1, 128], F32R, tag="cb")
                nc.vector.memset(cb.bitcast(F32), 1.0)
                # keep 1.0 where -k + q >= 0 (attend), else 0.0
                nc.gpsimd.affine_select(
                    out=cb.bitcast(F32)[:, 0, :],
                    in_=cb.bitcast(F32)[:, 0, :],
                    compare_op=mybir.AluOpType.is_ge, fill=0.0,
                    base=0, channel_multiplier=-1,
                    pattern=[[1, 128]])

            # ---- QKV projection ----
            # per-L-tile result tiles so attention for q-tile 0 can start
            # after 1/4 of the projection work; bufs=2 so rep r+1's
            # projection can fill fresh buffers while rep r's attention is
            # still reading the old ones (cross-rep pipelining)
            qkl = [const.tile([128, 4, 512], F32R, tag=f"qk{lt}",
                               name=f"qk{lt}", bufs=2)
                   for lt in range(NQT)]
            vtg = [const.tile([128, 4, HPC * (HD + 1)], F32R, tag=f"vt{g}",
                              name=f"vt{g}", bufs=2)
                   for g in range(NQT)]
            for g in range(NQT):
                # fill with 1.0; the v copies below overwrite all but the
                # per-head ones-columns (walrus rejects strided memsets)
                nc.vector.memset(vtg[g].bitcast(F32), 1.0)
            if True:

                def qkv_chunk(lt):
                    cp = nc.vector.tensor_copy
                    for g in range(4):     # interleave qk / v groups
                        ps = psum.tile([128, 512], F32, tag="qkps", bufs=1,
                                       name=f"qkps{lt}{g}")
                        xs = xslice(lt * 512, (lt + 1) * 512)
                        for kt in range(D // 128):
                            nc.tensor.matmul(
                                ps, wqkg[g][:, kt, :],
                                xs(kt),
                                start=(kt == 0), stop=(kt == D // 128 - 1))
                        cp(qkl[lt][:, g, :], ps)
                        l16 = 4 * lt + g
                        psv = psum.tile([128, DLOC], F32, tag="vps", bufs=1,
                                        name=f"vps{l16}")
                        xs = xslice(l16 * 128, (l16 + 1) * 128)
                        for kt in range(D // 128):
                            nc.tensor.matmul(
                                psv, xs(kt), wv[:, kt, :],
                                start=(kt == 0), stop=(kt == D // 128 - 1))
                        cp(vtg[lt][:, g, :]
                           .rearrange("p (h c) -> p h c", c=HD + 1)[:, :, 0:HD],
                           psv.rearrange("p (h c) -> p h c", c=HD))

                # ---- attention for one q-tile ----
                # Heads are processed in pairs (2hp, 2hp+1) living at
                # partition bases 0 / 64 of m-tile hp: their K=64 S^T matmuls
                # target disjoint PE row groups and run concurrently; exp is
                # fused over the pair ([128, 2, 512] per ACT op).
                at_tiles = {}

                def attention_unit(qt, hp):
                    # one attnT tile per head pair so the out-projection's
                    # p=0 matmuls can start while pair 1 still normalizes
                    if qt not in at_tiles:
                        at_tiles[qt] = [
                            atp.tile([128, 512], F32R, tag=f"at{p}",
                                     name=f"at{p}_{qt}") for p in range(2)]
                    ats = at_tiles[qt]
                    kts = [kt for kt in range(NKT) if status[kt, qt] != 0]
                    if True:
                        he, ho = 2 * hp, 2 * hp + 1
                        mq, mk = hp, 2 + hp
                        av = psum.tile([65, 2, 512], F32, tag="av", bufs=1,
                                       name=f"av{qt}{hp}")
                        for i, kt in enumerate(kts):
                            # causal mixed block at offset r: q-columns
                            # < 128r never attend this k-tile — shrink every
                            # op to the valid strip [c0:512] (the first kt of
                            # each q-tile is always full width, so the av
                            # accumulation bank is fully initialized)
                            mixed = status[kt, qt] == 2
                            c0 = 128 * (kt - 4 * qt) if (mixed and use_cb) \
                                else 0
                            st = psum.tile([128, 2, 512], F32, tag="st",
                                           bufs=2, name=f"st{qt}{hp}{kt}")
                            for j, base in ((0, 0), (1, 64)):
                                nc.tensor.matmul(
                                    st[:, j, c0:],
                                    qkl[kt // 4][base:base + 64, mk,
                                                 (kt % 4) * 128:
                                                 (kt % 4 + 1) * 128],
                                    qkl[qt][base:base + 64, mq, c0:],
                                    start=True, stop=True)
                            if mixed and not use_cb:
                                b_ap = misc.tile([128, 512], F32, tag="bt")
                                nc.sync.dma_start(
                                    out=b_ap,
                                    in_=bias_d.ap()[mixed_ids[(kt, qt)]])
                                for j in range(2):
                                    nc.vector.tensor_add(
                                        st[:, j, :], st[:, j, :], b_ap)
                            es = esp.tile([128, 2, 512], F32R, tag="es")
                            if "dveexp" in tweaks:
                                nc.vector.tensor_copy(
                                    es[:, :, c0:], st[:, :, c0:])
                            else:
                                nc.scalar.activation(es[:, :, c0:],
                                                     st[:, :, c0:], Exp)
                            if mixed and use_cb:
                                # only the 128-wide diagonal strip
                                # [c0, c0+128) is partial; it follows the
                                # r=0 triangle.  Columns < c0 are never read
                                # (every op above starts at c0), columns
                                # >= c0+128 attend fully.
                                nc.vector.tensor_mul(
                                    es[:, :, c0:c0 + 128],
                                    es[:, :, c0:c0 + 128],
                                    cb[:, 0:1, 0:128].broadcast_to(
                                        [128, 2, 128]))
                            for j, h in ((0, he), (1, ho)):
                                nc.tensor.matmul(
                                    av[:, j, c0:],
                                    vtg[kt // 4][:, kt % 4,
                                                 h * (HD + 1):(h + 1) * (HD + 1)],
                                    es[:, j, c0:],
                                    start=(i == 0), stop=(i == len(kts) - 1),
                                    skip_group_check=True)
                        # Free the av bank with one copy; normalize from the
                        # SBUF snapshot off the PE-critical path:
                        # attnT_h = av[0:64] / av[64]
                        # reciprocal must not be in-place (DVE in==out
                        # aliasing breaks on HW) and partition_broadcast's
                        # source must sit at partition 0 (the ucode reads
                        # physical partition 0, ignoring the AP offset)
                        avs = misc.tile([65, 2, 512], F32, tag="avs",
                                        bufs=1)
                        if "avsact" in tweaks:
                            # release the av bank via the ACT engine (slack
                            # between exps) instead of the congested DVE
                            # queue, so the next unit's av matmuls start
                            # sooner
                            nc.scalar.activation(
                                avs, av, mybir.ActivationFunctionType.Copy)
                        else:
                            nc.vector.tensor_copy(avs, av)
                        for j, base in ((0, 0), (1, 64)):
                            # pbcast's ucode reads from the physical tile
                            # start: give each j its own base-0 source tile
                            rc = misc.tile([1, 512], F32, tag=f"rc{j}",
                                           name=f"rc{j}", bufs=1)
                            nc.vector.reciprocal(rc, avs[64:65, j, :])
                            bc = misc.tile([64, 512], F32, tag="bc",
                                           bufs=1)
                            nc.gpsimd.partition_broadcast(bc, rc, channels=64)
                            nc.vector.tensor_mul(
                                ats[hp][base:base + 64, :],
                                avs[0:64, j, :], bc)

                def outproj_chunk(qt):
                    # out-projection for this q-tile.  Accumulates in an
                    # "st"-tagged PSUM pair (idle once this rep's attention
                    # is done) rather than qkps/vps, so the NEXT rep's QKV
                    # projection — which needs qkps/vps — isn't serialized
                    # behind this rep's final out-projection.
                    if qt not in at_tiles:   # "noattn" tweak: garbage attnT
                        at_tiles[qt] = [
                            atp.tile([128, 512], F32R, tag=f"at{p}",
                                     name=f"at{p}_{qt}") for p in range(2)]
                    ats = at_tiles[qt]
                    for lt in range(4):
                        row = qt * 512 + lt * 128
                        ot = otp.tile([128, 2, 512], F32, tag="ot")
                        if "opqk" in tweaks:
                            # original placement: borrow the projection's
                            # qkps/vps banks (A/B variant)
                            pos = [psum.tile([128, 512], F32, tag=t, bufs=1,
                                             name=f"po{qt}{lt}{t}")
                                   for t in ("qkps", "vps")]
                            for do in range(2):
                                for p in range(2):
                                    nc.tensor.matmul(
                                        pos[do],
                                        ats[p][:, lt * 128:(lt + 1) * 128],
                                        wo[:, p * D + do * 512:
                                           p * D + do * 512 + 512],
                                        start=(p == 0), stop=(p == 1))
                                nc.vector.tensor_copy(ot[:, do, :], pos[do])
                        else:
                            po = psum.tile([128, 2, 512], F32, tag="st",
                                           bufs=2, name=f"po{qt}{lt}")
                            for do in range(2):
                                for p in range(2):
                                    nc.tensor.matmul(
                                        po[:, do, :],
                                        ats[p][:, lt * 128:(lt + 1) * 128],
                                        wo[:, p * D + do * 512:
                                           p * D + do * 512 + 512],
                                        start=(p == 0), stop=(p == 1),
                                        skip_group_check=True)
                            nc.vector.tensor_copy(ot, po)
                        oeng = (nc.sync if ("dmabal" not in tweaks
                                            or (qt * 4 + lt) % 2 == 0)
                                else nc.scalar)
                        oeng.dma_start(
                            out=out_d.ap()[row:row + 128, :],
                            in_=ot.rearrange("p a b -> p (a b)"))

                if "noqkv" in tweaks:
                    qkv_chunk = lambda lt: None
                if "noattn" in tweaks:
                    attention_unit = lambda qt, hp: None
                if "noout" in tweaks:
                    outproj_chunk = lambda qt: None

                if use_cb:
                    # causal: q-tile qt only needs qkl/vtg up to chunk qt —
                    # stagger so exp/attention overlap the projection.  The
                    # qt>=2 attention tail is ACT(exp)-throughput-bound with
                    # little PE work of its own, so the PE-rich out-projection
                    # chunks are interleaved into it (tweak "tail0" keeps the
                    # old order: out0/out1 early, all outproj at the end)
                    qkv_chunk(0)
                    qkv_chunk(1)
                    attention_unit(0, 0)
                    attention_unit(0, 1)
                    if "tailmix" not in tweaks:
                        outproj_chunk(0)
                    qkv_chunk(2)
                    attention_unit(1, 0)
                    attention_unit(1, 1)
                    if "tailmix" not in tweaks:
                        outproj_chunk(1)
                    qkv_chunk(3)
                    if "tailmix" not in tweaks:
                        # measured on HW: keeping the out-projections early
                        # and the qt>=2 attention units pair-interleaved
                        # beats mixing outproj into the attention tail
                        # (st-tag PSUM ring contention), despite TimelineSim
                        # predicting the opposite
                        attention_unit(2, 0)
                        attention_unit(3, 0)
                        attention_unit(2, 1)
                        attention_unit(3, 1)
                        outproj_chunk(2)
                        outproj_chunk(3)
                    else:
                        attention_unit(2, 0)
                        outproj_chunk(0)
                        attention_unit(2, 1)
                        outproj_chunk(1)
                        attention_unit(3, 0)
                        outproj_chunk(2)
                        attention_unit(3, 1)
                        outproj_chunk(3)
                else:
                    for lt in range(NQT):
                        qkv_chunk(lt)
                    for qt in range(NQT):
                        attention_unit(qt, 0)
                        attention_unit(qt, 1)
                        outproj_chunk(qt)


def _host_prep(x, mask, w_qkv, w_out):
    x = np.ascontiguousarray(np.asarray(x, dtype=np.float32))
    mask = np.asarray(mask).astype(bool)
    w_qkv = np.asarray(w_qkv, dtype=np.float32)
    w_out = np.asarray(w_out, dtype=np.float32)

    tril = np.tril(np.ones((L, L), dtype=bool))
    is_causal = all(np.array_equal(mask[b], tril) for b in range(B))

    # block classification on the S^T layout: block (kt, qt) covers
    # k in [kt*128, ...), q in [qt*512, ...)
    status = np.zeros((NKT, NQT), np.int8)
    if is_causal:
        for qt in range(NQT):
            for kt in range(NKT):
                r = kt - 4 * qt
                status[kt, qt] = 0 if r > 3 else (2 if r >= 0 else 1)
    else:
        for qt in range(NQT):
            for kt in range(NKT):
                blk = mask[:, qt * 512:(qt + 1) * 512, kt * 128:(kt + 1) * 128]
                status[kt, qt] = 1 if blk.all() else (0 if not blk.any() else 2)

    # per-core inputs
    scale = float(HD) ** -0.5
    in_maps = []
    bias_by_batch = None
    if not is_causal:
        mixed = [(kt, qt) for qt in range(NQT) for kt in range(NKT)
                 if status[kt, qt] == 2]
        if mixed:
            bias_by_batch = []
            for b in range(B):
                tiles = np.zeros((len(mixed), 128, 512), np.float32)
                mt = mask[b].T  # [k, q]
                for i, (kt, qt) in enumerate(mixed):
                    blk = mt[kt * 128:(kt + 1) * 128, qt * 512:(qt + 1) * 512]
                    tiles[i] = np.where(blk, 0.0, NEG)
                bias_by_batch.append(tiles)

    for c in range(NCORES):
        b = c // CPB
        hq = (c % CPB) * HPC
        wq = w_qkv[hq * HD:(hq + HPC) * HD] * scale
        wk = w_qkv[D + hq * HD:D + (hq + HPC) * HD]
        wv = w_qkv[2 * D + hq * HD:2 * D + (hq + HPC) * HD]
        wqkT = np.ascontiguousarray(np.concatenate([wq, wk], 0).T)
        wvT = np.ascontiguousarray(wv.T)
        wo_loc = w_out[:, hq * HD:(hq + HPC) * HD].T       # [256, 1024]
        woT = np.ascontiguousarray(
            wo_loc.reshape(2, 128, D).transpose(1, 0, 2).reshape(128, 2 * D))
        im = {"xT": np.ascontiguousarray(x[b].T), "wqkT": wqkT,
              "wvT": wvT, "woT": woT}
        if bias_by_batch is not None:
            im["bias"] = bias_by_batch[b]
        in_maps.append(im)
    return status, is_causal, in_maps


LAST_RESULTS = None


def kernel(x, mask, w_qkv, w_out):
    from concourse.bass_utils import run_bass_kernel_spmd
    global LAST_RESULTS

    status, is_causal, in_maps = _host_prep(x, mask, w_qkv, w_out)
    key = (is_causal, status.tobytes(), 1)
    if key not in _built:
        _built[key] = _build(status, is_causal)
    nc = _built[key]

    res = run_bass_kernel_spmd(nc, in_maps, core_ids=list(range(NCORES)))
    LAST_RESULTS = res
    out = np.zeros((B, L, D), np.float64)
    for c in range(NCORES):
        out[c // CPB] += res.results[c]["out"].astype(np.float64)
    return out.astype(np.float32)


def make_runner(x, mask, w_qkv, w_out, reps=1):
    """Persistent jitted runner over 8 cores with device-resident inputs,
    for steady-state timing (mirrors bass2jax.run_bass_via_pjrt without
    output donation — this kernel writes every output element).

    reps>1 builds a program with the full kernel body repeated that many
    times back-to-back on device (each rep re-loads inputs from HBM and
    re-writes the full output), so one launch measures `reps` executions
    and the per-execution time isn't swamped by per-launch dispatch
    overhead.  The returned output is the last rep's (identical) result."""
    import jax
    import numpy as jnp_np
    from jax.sharding import Mesh, PartitionSpec, NamedSharding
    from jax.experimental.shard_map import shard_map
    from concourse import bass2jax
    import concourse.mybir as mybir

    bass2jax.install_neuronx_cc_hook()
    status, is_causal, in_maps = _host_prep(x, mask, w_qkv, w_out)
    key = (is_causal, status.tobytes(), reps)
    if key not in _built:
        _built[key] = _build(status, is_causal, reps=reps)
    nc = _built[key]

    partition_name = (nc.partition_id_tensor.name
                      if nc.partition_id_tensor else None)
    in_names, out_names, out_avals = [], [], []
    for alloc in nc.m.functions[0].allocations:
        if not isinstance(alloc, mybir.MemoryLocationSet):
            continue
        name = alloc.memorylocations[0].name
        if alloc.kind == "ExternalInput":
            if name != partition_name:
                in_names.append(name)
        elif alloc.kind == "ExternalOutput":
            out_names.append(name)
            out_avals.append(jax.core.ShapedArray(
                tuple(alloc.tensor_shape), mybir.dt.np(alloc.dtype)))
    n_params = len(in_names)
    all_in_names = in_names + out_names
    if partition_name is not None:
        all_in_names.append(partition_name)

    def _body(*args):
        operands = list(args)
        if partition_name is not None:
            operands.append(bass2jax.partition_id_tensor())
        outs = bass2jax._bass_exec_p.bind(
            *operands, out_avals=tuple(out_avals), in_names=tuple(all_in_names),
            out_names=tuple(out_names), lowering_input_output_aliases=(),
            sim_require_finite=True, sim_require_nnan=True, nc=nc)
        return tuple(outs)

    devices = jax.devices()[:NCORES]
    mesh = Mesh(np.asarray(devices), ("core",))
    spec = NamedSharding(mesh, PartitionSpec("core"))
    sharded = jax.jit(
        shard_map(_body, mesh=mesh,
                  in_specs=(PartitionSpec("core"),) * (n_params + len(out_names)),
                  out_specs=(PartitionSpec("core"),) * len(out_names),
                  check_rep=False),
        keep_unused=True)
    concat_in = [
        jax.device_put(
            np.concatenate([in_maps[c][n] for c in range(NCORES)], 0), spec)
        for n in in_names]
    concat_zeros = [
        jax.device_put(
            np.zeros((NCORES * a.shape[0], *a.shape[1:]), a.dtype), spec)
        for a in out_avals]

    def run():
        return sharded(*concat_in, *concat_zeros)

    def collect(out_arrs):
        full = np.asarray(out_arrs[0]).reshape(NCORES, L, D)
        out = np.zeros((B, L, D), np.float64)
        for c in range(NCORES):
            out[c // CPB] += full[c]
        return out.astype(np.float32)

    return run, collect

